# revision 10
# baseline (speedup 1.0000x reference)
"""Trainium2 Bass kernel for nn_BiGRUWithAttention.

Model: x -> BiGRU(128->512) -> BiGRU(1024->512) -> attn=tanh(h@Wa.T+ba) ->
       gated=attn*h -> out = gated@Wf.T+bf   (B=32, T=1024, out 10)

Sharding: 8 cores = 4 batch groups (8 samples each) x 2 directions.
Core c: dir d=c%2 (0=fwd, 1=bwd), group g=c//2. Replica pairs [[0,1],..].
The SPMD program is direction-agnostic: every core runs a *forward*
recurrence over its local time order tau (bwd cores get time-reversed
inputs prepared on the host). The fwd/bwd exchange between layers is an
AllGather of time-reversed hidden states plus a matmul-based selection
(host-supplied identity/zero matrices) so no core ever branches on its
rank.

Layouts (per core, everything "dims on partitions"):
  h_hist sbuf [128, 4*T*8] fp16   : col = k_block*T*8 + tau*8 + b
  gate psum  [128, 32] per gate   : M-tile j -> cols, block order r,n,z
  xg dram    [128, 12, T, 8] fp16 : precomputed input gates (bias folded)
"""
import sys, os
sys.path.insert(0, '/opt/trn_rl_repo')

import numpy as np
from contextlib import ExitStack

import concourse.bass as bass
import concourse.bacc as bacc
import concourse.tile as tile
from concourse import mybir
from concourse.bass_utils import run_bass_kernel_spmd

F16 = mybir.dt.float16
F32 = mybir.dt.float32
AF = mybir.ActivationFunctionType

N_CORES = 8
B, T_FULL, I_IN, H, O = 32, 1024, 128, 512, 10
G = 3 * H            # 1536 gate dims = 12 tiles of 128
BL = 8               # batch per core
# psum M-tile j -> row-block of W_hh/W_ih (gates stacked r,z,n in weights;
# psum layout r(j 0-3), n(j 4-7), z(j 8-11))
PERMROWS = [0, 1, 2, 3, 8, 9, 10, 11, 4, 5, 6, 7]
GROUPS = [[0, 1], [2, 3], [4, 5], [6, 7]]


# ----------------------------------------------------------------- program
def build_program(T=T_FULL, with_bhn=(False, False), with_bias=(False, False),
                  with_attn_bias=False, with_fc_bias=False):
    TH = T // 2
    NCOL = T * BL               # columns of the full sequence
    CH = min(512, NCOL)         # chunk width for big GEMM phases
    NCH = NCOL // CH
    NCOL2 = TH * BL             # attention token columns per core
    CH2 = min(512, NCOL2)
    NCH2 = NCOL2 // CH2
    XB = min(8, T)              # recurrence xg prefetch batch (steps)

    nc = bacc.Bacc("TRN2", target_bir_lowering=False, debug=False,
                   num_devices=N_CORES)

    def din(name, shape, dt=F16):
        return nc.dram_tensor(name, shape, dt, kind="ExternalInput").ap()

    xt = din("xt", [128, NCOL])                       # x.T (I on partitions)
    whh0 = din("whh0", [128, 48 * 128])
    whh1 = din("whh1", [128, 48 * 128])
    wih0 = din("wih0", [128, 12 * 128])
    wih1_own = din("wih1_own", [128, 48 * 128])
    wih1_oth = din("wih1_oth", [128, 48 * 128])
    sel0 = din("sel0", [128, 128])
    sel1 = din("sel1", [128, 128])
    attn_own = din("attn_own", [128, 32 * 128])
    attn_oth = din("attn_oth", [128, 32 * 128])
    fcw = din("fcw", [128, 8 * O])
    bias0 = din("bias0", [128, 12], F32)
    bias1 = din("bias1", [128, 12], F32)
    bhn0 = din("bhn0", [128, 32], F32)
    bhn1 = din("bhn1", [128, 32], F32)
    attn_b = din("attn_b", [128, 8], F32)
    fc_b = din("fc_b", [128, 1], F32)

    out_d = nc.dram_tensor("out", [O, TH, BL], F32, kind="ExternalOutput").ap()

    xg0d = nc.dram_tensor("xg0d", [128, 12, T, BL], F16).ap()
    xg1d = nc.dram_tensor("xg1d", [128, 12, T, BL], F16).ap()
    contrib0 = nc.dram_tensor("contrib0", [4, 128, T, BL], F16).ap()
    g0 = nc.dram_tensor("g0", [2, 4, 128, T, BL], F16).ap()
    contrib1 = nc.dram_tensor("contrib1", [4, 128, TH, BL], F16).ap()
    g1 = nc.dram_tensor("g1", [2, 4, 128, TH, BL], F16).ap()

    with ExitStack() as top:
        tc = top.enter_context(tile.TileContext(nc))

        const = top.enter_context(tc.tile_pool(name="const", bufs=1))
        # constants that live for the whole kernel
        sel0_sb = const.tile([128, 128], F16)
        sel1_sb = const.tile([128, 128], F16)
        nc.sync.dma_start(sel0_sb[:], sel0[:])
        nc.sync.dma_start(sel1_sb[:], sel1[:])

        # ---------------- phase helpers ----------------
        def xg_phase(ctx, wih_tiles, nk, rhs_of_k, xgd, bias_ap, namep):
            """xg[m] = sum_k W[m,k] @ rhs_k  (+bias) -> xgd dram (fp16)."""
            sb = ctx.enter_context(tc.tile_pool(name=namep + "sb", bufs=4))
            ps = ctx.enter_context(
                tc.tile_pool(name=namep + "ps", bufs=2, space="PSUM"))
            for c in range(NCH):
                for m in range(12):
                    p = ps.tile([128, CH], F32, tag="xgps")
                    for k in range(nk):
                        nc.tensor.matmul(
                            p[:], wih_tiles(m, k), rhs_of_k(k, c),
                            start=(k == 0), stop=(k == nk - 1))
                    o = sb.tile([128, CH], F16, tag="xgsb")
                    if bias_ap is not None:
                        if m % 2 == 0:
                            nc.scalar.activation(o[:], p[:], AF.Identity,
                                                 bias=bias_ap[:, m:m + 1])
                        else:
                            nc.vector.tensor_scalar_add(o[:], p[:],
                                                        bias_ap[:, m:m + 1])
                    else:
                        if m % 2 == 0:
                            nc.scalar.copy(o[:], p[:])
                        else:
                            nc.vector.tensor_copy(o[:], p[:])
                    t0 = c * (CH // BL)
                    t1 = (c + 1) * (CH // BL)
                    nc.sync.dma_start(xgd[:, m, t0:t1, :], o[:])

        def recurrence(ctx, xgd, whh_sb, h_hist, bhn_ap, namep):
            hr = h_hist[:].rearrange("p (k t b) -> p k t b", k=4, t=T)
            xgp = ctx.enter_context(tc.tile_pool(name=namep + "xg", bufs=3))
            tmp = ctx.enter_context(tc.tile_pool(name=namep + "tmp", bufs=3))
            hp = ctx.enter_context(tc.tile_pool(name=namep + "h32", bufs=2))
            psr = ctx.enter_context(
                tc.tile_pool(name=namep + "psr", bufs=2, space="PSUM"))
            psn = ctx.enter_context(
                tc.tile_pool(name=namep + "psn", bufs=2, space="PSUM"))
            psz = ctx.enter_context(
                tc.tile_pool(name=namep + "psz", bufs=2, space="PSUM"))
            zpool = ctx.enter_context(tc.tile_pool(name=namep + "z", bufs=1))

            zrhs = zpool.tile([128, 32], F16)
            nc.vector.memset(zrhs[:], 0.0)
            h32_prev = zpool.tile([128, 32], F32)
            nc.vector.memset(h32_prev[:], 0.0)

            xgc = None
            for t in range(T):
                if t % XB == 0:
                    xgc = xgp.tile([128, 12 * XB * BL], F16, tag="xgc")
                    nc.sync.dma_start(xgc[:], xgd[:, :, t:t + XB, :])
                xv = xgc[:].rearrange("p (m t b) -> p m t b", m=12, t=XB)
                tl = t % XB

                def rhs(k):
                    if t == 0:
                        return zrhs[:, k * 8:(k + 1) * 8]
                    return hr[:, k, t - 1, :]

                def mm(ps_t, j):
                    jo = j % 4
                    for k in range(4):
                        nc.tensor.matmul(
                            ps_t[:, jo * 8:(jo + 1) * 8],
                            whh_sb[:, (j * 4 + k) * 128:(j * 4 + k + 1) * 128],
                            rhs(k), start=(k == 0), stop=(k == 3))

                pr = psr.tile([128, 32], F32, tag="pr")
                for j in range(0, 4):
                    mm(pr, j)
                tr = tmp.tile([128, 32], F32, tag="tr")
                nc.vector.tensor_add(tr[:].rearrange("p (m b) -> p m b", m=4),
                                     pr[:].rearrange("p (m b) -> p m b", m=4),
                                     xv[:, 0:4, tl, :])
                rg = tmp.tile([128, 32], F32, tag="rg")
                nc.scalar.activation(rg[:], tr[:], AF.Sigmoid)

                pn = psn.tile([128, 32], F32, tag="pn")
                for j in range(4, 8):
                    mm(pn, j)
                if bhn_ap is not None:
                    t1_ = tmp.tile([128, 32], F32, tag="t1")
                    nc.vector.tensor_add(t1_[:], pn[:], bhn_ap)
                    nsrc = t1_
                else:
                    nsrc = pn
                t2 = tmp.tile([128, 32], F32, tag="t2")
                nc.vector.tensor_mul(t2[:], nsrc[:], rg[:])
                t3 = tmp.tile([128, 32], F32, tag="t3")
                nc.vector.tensor_add(t3[:].rearrange("p (m b) -> p m b", m=4),
                                     t2[:].rearrange("p (m b) -> p m b", m=4),
                                     xv[:, 4:8, tl, :])
                ng = tmp.tile([128, 32], F32, tag="ng")
                nc.scalar.activation(ng[:], t3[:], AF.Tanh)

                pz = psz.tile([128, 32], F32, tag="pz")
                for j in range(8, 12):
                    mm(pz, j)
                tz = tmp.tile([128, 32], F32, tag="tz")
                nc.vector.tensor_add(tz[:].rearrange("p (m b) -> p m b", m=4),
                                     pz[:].rearrange("p (m b) -> p m b", m=4),
                                     xv[:, 8:12, tl, :])
                zg = tmp.tile([128, 32], F32, tag="zg")
                nc.scalar.activation(zg[:], tz[:], AF.Sigmoid)

                dd = tmp.tile([128, 32], F32, tag="dd")
                nc.vector.tensor_sub(dd[:], h32_prev[:], ng[:])
                ee = tmp.tile([128, 32], F32, tag="ee")
                nc.vector.tensor_mul(ee[:], zg[:], dd[:])
                h32 = hp.tile([128, 32], F32, tag="h32")
                nc.vector.tensor_add(h32[:], ng[:], ee[:])
                nc.vector.tensor_copy(
                    hr[:, :, t, :],
                    h32[:].rearrange("p (k b) -> p k b", k=4))
                h32_prev = h32

        def exchange(h_hist, t_lo, t_cnt, contrib, gbuf):
            """contrib[i] = h_hist[t_lo + t_cnt-1 - i]; AllGather -> gbuf."""
            hr = h_hist[:].rearrange("p (k t b) -> p k t b", k=4, t=T)
            tch = min(256, t_cnt)
            for k in range(4):
                src = hr[:, k, t_lo:t_lo + t_cnt, :][:, ::-1, :]
                for q in range(t_cnt // tch):
                    nc.sync.dma_start(contrib[k, :, q * tch:(q + 1) * tch, :],
                                      src[:, q * tch:(q + 1) * tch, :])
            nc.gpsimd.collective_compute(
                "AllGather", mybir.AluOpType.bypass,
                ins=[contrib[:]], outs=[gbuf[:]], replica_groups=GROUPS)

        def sel_other(ctx_pools, gbuf, tcols, c, ch):
            """Select other-direction k-blocks from gathered buffer, chunk c."""
            selsb, selps, hoth_pool = ctx_pools
            t0 = c * (ch // BL)
            t1 = t0 + ch // BL
            hoth = []
            for kb in range(4):
                s0 = selsb.tile([128, ch], F16, tag="s0")
                nc.sync.dma_start(s0[:], gbuf[0, kb, :, t0:t1, :])
                s1 = selsb.tile([128, ch], F16, tag="s1")
                nc.sync.dma_start(s1[:], gbuf[1, kb, :, t0:t1, :])
                p = selps.tile([128, ch], F32, tag="selps")
                nc.tensor.matmul(p[:], sel0_sb[:], s0[:], start=True, stop=False)
                nc.tensor.matmul(p[:], sel1_sb[:], s1[:], start=False, stop=True)
                ho = hoth_pool.tile([128, ch], F16, tag="hoth")
                nc.vector.tensor_copy(ho[:], p[:])
                hoth.append(ho)
            return hoth

        # ---------------- phase 1: xg0 ----------------
        with ExitStack() as ctx:
            xsb = ctx.enter_context(tc.tile_pool(name="xsb", bufs=1))
            x_sb = xsb.tile([128, NCOL], F16)
            nc.sync.dma_start(x_sb[:], xt[:])
            wp = ctx.enter_context(tc.tile_pool(name="wih0p", bufs=1))
            wih0_sb = wp.tile([128, 12 * 128], F16)
            nc.sync.dma_start(wih0_sb[:], wih0[:])
            if with_bias[0]:
                b0p = ctx.enter_context(tc.tile_pool(name="b0p", bufs=1))
                b0_sb = b0p.tile([128, 12], F32)
                nc.sync.dma_start(b0_sb[:], bias0[:])
                b0_ap = b0_sb[:]
            else:
                b0_ap = None
            xg_phase(ctx,
                     lambda m, k: wih0_sb[:, m * 128:(m + 1) * 128],
                     1,
                     lambda k, c: x_sb[:, c * CH:(c + 1) * CH],
                     xg0d, b0_ap, "x0")

        # ---------------- phase 2: L0 recurrence ----------------
        h0_scope = ExitStack()
        h0p = h0_scope.enter_context(tc.tile_pool(name="h0p", bufs=1))
        h0_hist = h0p.tile([128, 4 * T * BL], F16)
        with ExitStack() as ctx:
            wp = ctx.enter_context(tc.tile_pool(name="whh0p", bufs=1))
            whh0_sb = wp.tile([128, 48 * 128], F16)
            nc.sync.dma_start(whh0_sb[:], whh0[:])
            bz = ctx.enter_context(tc.tile_pool(name="bhn0p", bufs=1))
            if with_bhn[0]:
                bhn0_sb = bz.tile([128, 32], F32)
                nc.sync.dma_start(bhn0_sb[:], bhn0[:])
                bhn_ap = bhn0_sb[:]
            else:
                bhn_ap = None
            recurrence(ctx, xg0d, whh0_sb, h0_hist, bhn_ap, "r0")

        # ---------------- phase 3: exchange h0 ----------------
        exchange(h0_hist, 0, T, contrib0, g0)

        # ---------------- phase 4: xg1 ----------------
        with ExitStack() as ctx:
            wp = ctx.enter_context(tc.tile_pool(name="wih1p", bufs=1))
            wih1o_sb = wp.tile([128, 48 * 128], F16, tag="wo")
            nc.sync.dma_start(wih1o_sb[:], wih1_own[:])
            wih1x_sb = wp.tile([128, 48 * 128], F16, tag="wx")
            nc.sync.dma_start(wih1x_sb[:], wih1_oth[:])
            b1p = ctx.enter_context(tc.tile_pool(name="b1p", bufs=1))
            if with_bias[1]:
                b1_sb = b1p.tile([128, 12], F32)
                nc.sync.dma_start(b1_sb[:], bias1[:])
                b1_ap = b1_sb[:]
            else:
                b1_ap = None
            selsb = ctx.enter_context(tc.tile_pool(name="sl4", bufs=3))
            selps = ctx.enter_context(
                tc.tile_pool(name="slp4", bufs=2, space="PSUM"))
            hop = ctx.enter_context(tc.tile_pool(name="ho4", bufs=8))
            sb = ctx.enter_context(tc.tile_pool(name="x1sb", bufs=4))
            ps = ctx.enter_context(
                tc.tile_pool(name="x1ps", bufs=2, space="PSUM"))
            h0r_flat = h0_hist
            for c in range(NCH):
                hoth = sel_other((selsb, selps, hop), g0, T, c, CH)
                for m in range(12):
                    p = ps.tile([128, CH], F32, tag="x1p")
                    for k in range(4):
                        nc.tensor.matmul(
                            p[:],
                            wih1o_sb[:, (m * 4 + k) * 128:(m * 4 + k + 1) * 128],
                            h0r_flat[:, k * NCOL + c * CH: k * NCOL + (c + 1) * CH],
                            start=(k == 0), stop=False)
                    for k in range(4):
                        nc.tensor.matmul(
                            p[:],
                            wih1x_sb[:, (m * 4 + k) * 128:(m * 4 + k + 1) * 128],
                            hoth[k][:], start=False, stop=(k == 3))
                    o = sb.tile([128, CH], F16, tag="x1o")
                    if b1_ap is not None:
                        if m % 2 == 0:
                            nc.scalar.activation(o[:], p[:], AF.Identity,
                                                 bias=b1_ap[:, m:m + 1])
                        else:
                            nc.vector.tensor_scalar_add(o[:], p[:],
                                                        b1_ap[:, m:m + 1])
                    else:
                        if m % 2 == 0:
                            nc.scalar.copy(o[:], p[:])
                        else:
                            nc.vector.tensor_copy(o[:], p[:])
                    t0 = c * (CH // BL)
                    t1 = (c + 1) * (CH // BL)
                    nc.sync.dma_start(xg1d[:, m, t0:t1, :], o[:])
        h0_scope.close()

        # ---------------- phase 5: L1 recurrence ----------------
        h1_scope = ExitStack()
        h1p = h1_scope.enter_context(tc.tile_pool(name="h1p", bufs=1))
        h1_hist = h1p.tile([128, 4 * T * BL], F16)
        with ExitStack() as ctx:
            wp = ctx.enter_context(tc.tile_pool(name="whh1p", bufs=1))
            whh1_sb = wp.tile([128, 48 * 128], F16)
            nc.sync.dma_start(whh1_sb[:], whh1[:])
            bz = ctx.enter_context(tc.tile_pool(name="bhn1p", bufs=1))
            if with_bhn[1]:
                bhn1_sb = bz.tile([128, 32], F32)
                nc.sync.dma_start(bhn1_sb[:], bhn1[:])
                bhn_ap = bhn1_sb[:]
            else:
                bhn_ap = None
            recurrence(ctx, xg1d, whh1_sb, h1_hist, bhn_ap, "r1")

        # ---------------- phase 6: exchange h1 tail ----------------
        exchange(h1_hist, TH, TH, contrib1, g1)

        # ---------------- phase 7: attention + fc ----------------
        with ExitStack() as ctx:
            wp = ctx.enter_context(tc.tile_pool(name="awp", bufs=1))
            attno_sb = wp.tile([128, 32 * 128], F16, tag="ao")
            nc.sync.dma_start(attno_sb[:], attn_own[:])
            attnx_sb = wp.tile([128, 32 * 128], F16, tag="ax")
            nc.sync.dma_start(attnx_sb[:], attn_oth[:])
            fcw_sb = wp.tile([128, 8 * O], F16, tag="fw")
            nc.sync.dma_start(fcw_sb[:], fcw[:])
            ab_sb = wp.tile([128, 8], F32, tag="ab")
            if with_attn_bias:
                nc.sync.dma_start(ab_sb[:], attn_b[:])
            fb_sb = wp.tile([128, 1], F32, tag="fb")
            if with_fc_bias:
                nc.sync.dma_start(fb_sb[:], fc_b[:])

            selsb = ctx.enter_context(tc.tile_pool(name="sl7", bufs=3))
            selps = ctx.enter_context(
                tc.tile_pool(name="slp7", bufs=2, space="PSUM"))
            hop = ctx.enter_context(tc.tile_pool(name="ho7", bufs=8))
            sb = ctx.enter_context(tc.tile_pool(name="asb", bufs=4))
            aps = ctx.enter_context(
                tc.tile_pool(name="aps", bufs=2, space="PSUM"))
            fps = ctx.enter_context(
                tc.tile_pool(name="fps", bufs=2, space="PSUM"))
            for c in range(NCH2):
                hoth = sel_other((selsb, selps, hop), g1, TH, c, CH2)
                pf = fps.tile([O, CH2], F32, tag="fcp")
                for m in range(8):
                    p = aps.tile([128, CH2], F32, tag="ap")
                    for k in range(4):
                        nc.tensor.matmul(
                            p[:],
                            attno_sb[:, (m * 4 + k) * 128:(m * 4 + k + 1) * 128],
                            h1_hist[:, k * NCOL + c * CH2: k * NCOL + (c + 1) * CH2],
                            start=(k == 0), stop=False)
                    for k in range(4):
                        nc.tensor.matmul(
                            p[:],
                            attnx_sb[:, (m * 4 + k) * 128:(m * 4 + k + 1) * 128],
                            hoth[k][:], start=False, stop=(k == 3))
                    at = sb.tile([128, CH2], F32, tag="at")
                    if with_attn_bias:
                        nc.scalar.activation(at[:], p[:], AF.Tanh,
                                             bias=ab_sb[:, m:m + 1])
                    else:
                        nc.scalar.activation(at[:], p[:], AF.Tanh)
                    gt = sb.tile([128, CH2], F16, tag="gt")
                    if m < 4:
                        hloc = h1_hist[:, m * NCOL + c * CH2: m * NCOL + (c + 1) * CH2]
                    else:
                        hloc = hoth[m - 4][:]
                    nc.vector.tensor_mul(gt[:], at[:], hloc)
                    nc.tensor.matmul(pf[:], fcw_sb[:, m * O:(m + 1) * O], gt[:],
                                     start=(m == 0), stop=(m == 7))
                ot = sb.tile([O, CH2], F32, tag="ot")
                if with_fc_bias:
                    nc.scalar.activation(ot[:], pf[:], AF.Identity,
                                         bias=fb_sb[0:O, 0:1])
                else:
                    nc.scalar.copy(ot[:], pf[:])
                t0 = c * (CH2 // BL)
                t1 = (c + 1) * (CH2 // BL)
                nc.sync.dma_start(out_d[:, t0:t1, :], ot[:])
        h1_scope.close()

    nc.compile()
    return nc


# ----------------------------------------------------------------- host prep
def prep_core_inputs(inputs, c, T=T_FULL):
    d, g = c % 2, c // 2
    TH = T // 2
    f16 = lambda a: np.ascontiguousarray(a, dtype=np.float16)
    f32 = lambda a: np.ascontiguousarray(a, dtype=np.float32)

    x = np.asarray(inputs['x'])[g * BL:(g + 1) * BL, :T]      # [8, T, 128]
    if d == 1:
        x = x[:, ::-1]
    xt = f16(x.transpose(2, 1, 0).reshape(128, T * BL))

    w_hh0 = np.asarray(inputs['W_hh0'])[d]     # [1536, 512]
    w_hh1 = np.asarray(inputs['W_hh1'])[d]
    w_ih0 = np.asarray(inputs['W_ih0'])[d]     # [1536, 128]
    w_ih1 = np.asarray(inputs['W_ih1'])[d]     # [1536, 1024]
    b_ih0 = np.asarray(inputs['b_ih0'])[d]
    b_hh0 = np.asarray(inputs['b_hh0'])[d]
    b_ih1 = np.asarray(inputs['b_ih1'])[d]
    b_hh1 = np.asarray(inputs['b_hh1'])[d]
    attn_W = np.asarray(inputs['attn_W'])      # [1024, 1024]
    attn_bv = np.asarray(inputs['attn_b'])
    fc_W = np.asarray(inputs['fc_W'])          # [10, 1024]
    fc_bv = np.asarray(inputs['fc_b'])

    def whh_tiles(w):
        out = np.zeros((128, 48 * 128), np.float16)
        for j in range(12):
            rb = PERMROWS[j]
            for k in range(4):
                blk = w[rb * 128:(rb + 1) * 128, k * 128:(k + 1) * 128]
                out[:, (j * 4 + k) * 128:(j * 4 + k + 1) * 128] = \
                    blk.T.astype(np.float16)
        return out

    whh0 = whh_tiles(w_hh0)
    whh1 = whh_tiles(w_hh1)

    wih0 = np.zeros((128, 12 * 128), np.float16)
    for j in range(12):
        rb = PERMROWS[j]
        wih0[:, j * 128:(j + 1) * 128] = \
            w_ih0[rb * 128:(rb + 1) * 128, :].T.astype(np.float16)

    own_lo = 0 if d == 0 else 512
    oth_lo = 512 - own_lo

    def wih1_tiles(col_lo):
        out = np.zeros((128, 48 * 128), np.float16)
        for j in range(12):
            rb = PERMROWS[j]
            for k in range(4):
                blk = w_ih1[rb * 128:(rb + 1) * 128,
                            col_lo + k * 128: col_lo + (k + 1) * 128]
                out[:, (j * 4 + k) * 128:(j * 4 + k + 1) * 128] = \
                    blk.T.astype(np.float16)
        return out

    wih1_own = wih1_tiles(own_lo)
    wih1_oth = wih1_tiles(oth_lo)

    ident = np.eye(128, dtype=np.float16)
    zer = np.zeros((128, 128), np.float16)
    sel0 = ident if d == 1 else zer      # gathered rank0 = fwd core
    sel1 = ident if d == 0 else zer

    # attention: rows and cols in LOCAL order (own dims first)
    attn_local = np.concatenate(
        [attn_W[own_lo:own_lo + 512], attn_W[oth_lo:oth_lo + 512]], axis=0)

    def attn_tiles(col_lo):
        out = np.zeros((128, 32 * 128), np.float16)
        for m in range(8):
            for k in range(4):
                blk = attn_local[m * 128:(m + 1) * 128,
                                 col_lo + k * 128: col_lo + (k + 1) * 128]
                out[:, (m * 4 + k) * 128:(m * 4 + k + 1) * 128] = \
                    blk.T.astype(np.float16)
        return out

    attn_own = attn_tiles(own_lo)
    attn_oth = attn_tiles(oth_lo)

    fc_local = np.concatenate(
        [fc_W[:, own_lo:own_lo + 512], fc_W[:, oth_lo:oth_lo + 512]], axis=1)
    fcw = np.zeros((128, 8 * O), np.float16)
    for k in range(8):
        fcw[:, k * O:(k + 1) * O] = \
            fc_local[:, k * 128:(k + 1) * 128].T.astype(np.float16)

    # biases: fold b_ih + b_hh(r,z) into xg bias; n keeps b_ih only + bhn tile
    def gate_bias(b_ih, b_hh):
        v = b_ih.astype(np.float64).copy()
        v[:H] += b_hh[:H]              # r
        v[H:2 * H] += b_hh[H:2 * H]    # z
        bias = np.zeros((128, 12), np.float32)
        for j in range(12):
            rb = PERMROWS[j]
            bias[:, j] = v[rb * 128:(rb + 1) * 128]
        return bias

    bias0 = gate_bias(b_ih0, b_hh0)
    bias1 = gate_bias(b_ih1, b_hh1)
    bhn0 = np.zeros((128, 32), np.float32)
    bhn1 = np.zeros((128, 32), np.float32)
    for jj in range(4):
        bhn0[:, jj * 8:(jj + 1) * 8] = \
            b_hh0[2 * H + jj * 128: 2 * H + (jj + 1) * 128, None]
        bhn1[:, jj * 8:(jj + 1) * 8] = \
            b_hh1[2 * H + jj * 128: 2 * H + (jj + 1) * 128, None]

    attn_b_local = np.concatenate(
        [attn_bv[own_lo:own_lo + 512], attn_bv[oth_lo:oth_lo + 512]])
    attn_b = np.zeros((128, 8), np.float32)
    for m in range(8):
        attn_b[:, m] = attn_b_local[m * 128:(m + 1) * 128]
    fc_b = np.zeros((128, 1), np.float32)
    fc_b[:O, 0] = fc_bv

    return {
        "xt": xt, "whh0": whh0, "whh1": whh1, "wih0": wih0,
        "wih1_own": wih1_own, "wih1_oth": wih1_oth,
        "sel0": sel0, "sel1": sel1,
        "attn_own": attn_own, "attn_oth": attn_oth, "fcw": fcw,
        "bias0": f32(bias0), "bias1": f32(bias1),
        "bhn0": f32(bhn0), "bhn1": f32(bhn1),
        "attn_b": f32(attn_b), "fc_b": f32(fc_b),
    }


def flags_from_inputs(inputs):
    nz = lambda a: bool(np.any(np.asarray(a)))
    with_bhn = (nz(np.asarray(inputs['b_hh0'])[:, 2 * H:]),
                nz(np.asarray(inputs['b_hh1'])[:, 2 * H:]))
    with_bias = (nz(inputs['b_ih0']) or nz(np.asarray(inputs['b_hh0'])[:, :2 * H]),
                 nz(inputs['b_ih1']) or nz(np.asarray(inputs['b_hh1'])[:, :2 * H]))
    return dict(with_bhn=with_bhn, with_bias=with_bias,
                with_attn_bias=nz(inputs['attn_b']),
                with_fc_bias=nz(inputs['fc_b']))


_PROG_CACHE = {}


def _get_program(T, flags):
    key = (T, tuple(sorted((k, tuple(v) if isinstance(v, tuple) else v)
                           for k, v in flags.items())))
    if key not in _PROG_CACHE:
        _PROG_CACHE[key] = build_program(T=T, **flags)
    return _PROG_CACHE[key]


def run_cores(inputs, T=T_FULL, trace=False):
    flags = flags_from_inputs(inputs)
    nc = _get_program(T, flags)
    in_maps = [prep_core_inputs(inputs, c, T=T) for c in range(N_CORES)]
    res = run_bass_kernel_spmd(nc, in_maps, list(range(N_CORES)), trace=trace)
    return res


def assemble_output(results, T=T_FULL):
    TH = T // 2
    out = np.zeros((B, T, O), np.float32)
    for c in range(N_CORES):
        d, g = c % 2, c // 2
        r = results[c]["out"].transpose(2, 1, 0)   # [O,TH,BL] -> [BL,TH,O]
        if d == 0:
            out[g * BL:(g + 1) * BL, :TH] = r
        else:
            out[g * BL:(g + 1) * BL, TH:] = r[:, ::-1, :]
    return out


def kernel(**inputs) -> np.ndarray:
    res = run_cores(inputs, T=T_FULL)
    return assemble_output(res.results, T=T_FULL)


if __name__ == "__main__":
    pass


# revision 11
# speedup vs baseline: 6.2722x; 6.2722x over previous
"""Trainium2 Bass kernel for nn_BiGRUWithAttention.

Model: x -> BiGRU(128->512) -> BiGRU(1024->512) -> attn=tanh(h@Wa.T+ba) ->
       gated=attn*h -> out = gated@Wf.T+bf   (B=32, T=1024, out 10)

Sharding: 8 cores = 4 batch groups (8 samples each) x 2 directions.
Core c: dir d=c%2 (0=fwd, 1=bwd), group g=c//2. Replica pairs [[0,1],..].
The SPMD program is direction-agnostic: every core runs a *forward*
recurrence over its local time order tau (bwd cores get time-reversed
inputs prepared on the host). The fwd/bwd exchange between layers is an
AllGather of time-reversed hidden states plus a matmul-based selection
(host-supplied identity/zero matrices) so no core ever branches on its
rank.

Layouts (per core, everything "dims on partitions"):
  h_hist sbuf [128, 4*T*8] fp16   : col = k_block*T*8 + tau*8 + b
  gate psum  [128, 32] per gate   : M-tile j -> cols, block order r,n,z
  xg dram    [128, 12, T, 8] fp16 : precomputed input gates (bias folded)
"""
import sys, os
sys.path.insert(0, '/opt/trn_rl_repo')

import numpy as np
from contextlib import ExitStack

import concourse.bass as bass
import concourse.bacc as bacc
import concourse.tile as tile
from concourse import mybir
from concourse.bass_utils import run_bass_kernel_spmd

F16 = mybir.dt.float16
F32 = mybir.dt.float32
AF = mybir.ActivationFunctionType

N_CORES = 8
B, T_FULL, I_IN, H, O = 32, 1024, 128, 512, 10
G = 3 * H            # 1536 gate dims = 12 tiles of 128
BL = 8               # batch per core
# psum M-tile j -> row-block of W_hh/W_ih (gates stacked r,z,n in weights;
# psum layout r(j 0-3), n(j 4-7), z(j 8-11))
PERMROWS = [0, 1, 2, 3, 8, 9, 10, 11, 4, 5, 6, 7]
GROUPS = [[0, 1], [2, 3], [4, 5], [6, 7]]


# ----------------------------------------------------------------- program
def build_program(T=T_FULL, with_bhn=(False, False), with_bias=(False, False),
                  with_attn_bias=False, with_fc_bias=False):
    TH = T // 2
    NCOL = T * BL               # columns of the full sequence
    CH = min(512, NCOL)         # chunk width for big GEMM phases
    NCH = NCOL // CH
    NCOL2 = TH * BL             # attention token columns per core
    CH2 = min(512, NCOL2)
    NCH2 = NCOL2 // CH2
    XB = min(8, T)              # recurrence xg prefetch batch (steps)

    nc = bacc.Bacc("TRN2", target_bir_lowering=False, debug=False,
                   num_devices=N_CORES)

    def din(name, shape, dt=F16):
        return nc.dram_tensor(name, shape, dt, kind="ExternalInput").ap()

    xt = din("xt", [128, NCOL])                       # x.T (I on partitions)
    whh0 = din("whh0", [128, 48 * 128])
    whh1 = din("whh1", [128, 48 * 128])
    wih0 = din("wih0", [128, 12 * 128])
    wih1_own = din("wih1_own", [128, 48 * 128])
    wih1_oth = din("wih1_oth", [128, 48 * 128])
    sel0 = din("sel0", [128, 128])
    sel1 = din("sel1", [128, 128])
    attn_own = din("attn_own", [128, 32 * 128])
    attn_oth = din("attn_oth", [128, 32 * 128])
    fcw = din("fcw", [128, 8 * O])
    bias0 = din("bias0", [128, 12], F32)
    bias1 = din("bias1", [128, 12], F32)
    bhn0 = din("bhn0", [128, 32], F32)
    bhn1 = din("bhn1", [128, 32], F32)
    attn_b = din("attn_b", [128, 8], F32)
    fc_b = din("fc_b", [128, 1], F32)

    out_d = nc.dram_tensor("out", [O, TH, BL], F32, kind="ExternalOutput").ap()

    xg0d = nc.dram_tensor("xg0d", [128, 12, T, BL], F16).ap()
    xg1d = nc.dram_tensor("xg1d", [128, 12, T, BL], F16).ap()
    contrib0 = nc.dram_tensor("contrib0", [4, 128, T, BL], F16).ap()
    g0 = nc.dram_tensor("g0", [2, 4, 128, T, BL], F16).ap()
    contrib1 = nc.dram_tensor("contrib1", [4, 128, TH, BL], F16).ap()
    g1 = nc.dram_tensor("g1", [2, 4, 128, TH, BL], F16).ap()

    with ExitStack() as top:
        tc = top.enter_context(tile.TileContext(nc))

        const = top.enter_context(tc.tile_pool(name="const", bufs=1))
        # constants that live for the whole kernel
        sel0_sb = const.tile([128, 128], F16)
        sel1_sb = const.tile([128, 128], F16)
        nc.sync.dma_start(sel0_sb[:], sel0[:])
        nc.sync.dma_start(sel1_sb[:], sel1[:])

        # ---------------- phase helpers ----------------
        def xg_phase(ctx, wih_tiles, nk, rhs_of_k, xgd, bias_ap, namep):
            """xg[m] = sum_k W[m,k] @ rhs_k  (+bias) -> xgd dram (fp16)."""
            sb = ctx.enter_context(tc.tile_pool(name=namep + "sb", bufs=4))
            ps = ctx.enter_context(
                tc.tile_pool(name=namep + "ps", bufs=2, space="PSUM"))
            for c in range(NCH):
                for m in range(12):
                    p = ps.tile([128, CH], F32, tag="xgps")
                    for k in range(nk):
                        nc.tensor.matmul(
                            p[:], wih_tiles(m, k), rhs_of_k(k, c),
                            start=(k == 0), stop=(k == nk - 1))
                    o = sb.tile([128, CH], F16, tag="xgsb")
                    if bias_ap is not None:
                        if m % 2 == 0:
                            nc.scalar.activation(o[:], p[:], AF.Identity,
                                                 bias=bias_ap[:, m:m + 1])
                        else:
                            nc.vector.tensor_scalar_add(o[:], p[:],
                                                        bias_ap[:, m:m + 1])
                    else:
                        if m % 2 == 0:
                            nc.scalar.copy(o[:], p[:])
                        else:
                            nc.vector.tensor_copy(o[:], p[:])
                    t0 = c * (CH // BL)
                    t1 = (c + 1) * (CH // BL)
                    nc.sync.dma_start(xgd[:, m, t0:t1, :], o[:])

        def recurrence(ctx, xgd, whh_sb, h_hist, bhn_ap, namep):
            hr = h_hist[:].rearrange("p (k t b) -> p k t b", k=4, t=T)
            xgp = ctx.enter_context(tc.tile_pool(name=namep + "xg", bufs=3))
            tmp = ctx.enter_context(tc.tile_pool(name=namep + "tmp", bufs=3))
            hp = ctx.enter_context(tc.tile_pool(name=namep + "h32", bufs=2))
            psr = ctx.enter_context(
                tc.tile_pool(name=namep + "psr", bufs=2, space="PSUM"))
            psn = ctx.enter_context(
                tc.tile_pool(name=namep + "psn", bufs=2, space="PSUM"))
            psz = ctx.enter_context(
                tc.tile_pool(name=namep + "psz", bufs=2, space="PSUM"))
            zpool = ctx.enter_context(tc.tile_pool(name=namep + "z", bufs=1))

            zrhs = zpool.tile([128, 32], F16)
            nc.vector.memset(zrhs[:], 0.0)
            h32_prev = zpool.tile([128, 32], F32)
            nc.vector.memset(h32_prev[:], 0.0)

            xgc = None
            for t in range(T):
                if t % XB == 0:
                    xgc = xgp.tile([128, 12 * XB * BL], F16, tag="xgc")
                    nc.sync.dma_start(xgc[:], xgd[:, :, t:t + XB, :])
                xv = xgc[:].rearrange("p (m t b) -> p m t b", m=12, t=XB)
                tl = t % XB

                def rhs(k):
                    if t == 0:
                        return zrhs[:, k * 8:(k + 1) * 8]
                    return hr[:, k, t - 1, :]

                def mm(ps_t, j):
                    jo = j % 4
                    for k in range(4):
                        nc.tensor.matmul(
                            ps_t[:, jo * 8:(jo + 1) * 8],
                            whh_sb[:, (j * 4 + k) * 128:(j * 4 + k + 1) * 128],
                            rhs(k), start=(k == 0), stop=(k == 3))

                pr = psr.tile([128, 32], F32, tag="pr")
                for j in range(0, 4):
                    mm(pr, j)
                tr = tmp.tile([128, 32], F32, tag="tr")
                nc.vector.tensor_add(tr[:].rearrange("p (m b) -> p m b", m=4),
                                     pr[:].rearrange("p (m b) -> p m b", m=4),
                                     xv[:, 0:4, tl, :])
                rg = tmp.tile([128, 32], F32, tag="rg")
                nc.scalar.activation(rg[:], tr[:], AF.Sigmoid)

                pn = psn.tile([128, 32], F32, tag="pn")
                for j in range(4, 8):
                    mm(pn, j)
                if bhn_ap is not None:
                    t1_ = tmp.tile([128, 32], F32, tag="t1")
                    nc.vector.tensor_add(t1_[:], pn[:], bhn_ap)
                    nsrc = t1_
                else:
                    nsrc = pn
                t2 = tmp.tile([128, 32], F32, tag="t2")
                nc.vector.tensor_mul(t2[:], nsrc[:], rg[:])
                t3 = tmp.tile([128, 32], F32, tag="t3")
                nc.vector.tensor_add(t3[:].rearrange("p (m b) -> p m b", m=4),
                                     t2[:].rearrange("p (m b) -> p m b", m=4),
                                     xv[:, 4:8, tl, :])
                ng = tmp.tile([128, 32], F32, tag="ng")
                nc.scalar.activation(ng[:], t3[:], AF.Tanh)

                pz = psz.tile([128, 32], F32, tag="pz")
                for j in range(8, 12):
                    mm(pz, j)
                tz = tmp.tile([128, 32], F32, tag="tz")
                nc.vector.tensor_add(tz[:].rearrange("p (m b) -> p m b", m=4),
                                     pz[:].rearrange("p (m b) -> p m b", m=4),
                                     xv[:, 8:12, tl, :])
                zg = tmp.tile([128, 32], F32, tag="zg")
                nc.scalar.activation(zg[:], tz[:], AF.Sigmoid)

                dd = tmp.tile([128, 32], F32, tag="dd")
                nc.vector.tensor_sub(dd[:], h32_prev[:], ng[:])
                ee = tmp.tile([128, 32], F32, tag="ee")
                nc.vector.tensor_mul(ee[:], zg[:], dd[:])
                h32 = hp.tile([128, 32], F32, tag="h32")
                nc.vector.tensor_add(h32[:], ng[:], ee[:])
                nc.vector.tensor_copy(
                    hr[:, :, t, :],
                    h32[:].rearrange("p (k b) -> p k b", k=4))
                h32_prev = h32

        def exchange(h_hist, t_lo, t_cnt, contrib, gbuf):
            """contrib[i] = h_hist[t_lo + t_cnt-1 - i]; AllGather -> gbuf."""
            hr = h_hist[:].rearrange("p (k t b) -> p k t b", k=4, t=T)
            tch = min(256, t_cnt)
            for k in range(4):
                src = hr[:, k, t_lo:t_lo + t_cnt, :][:, ::-1, :]
                for q in range(t_cnt // tch):
                    nc.sync.dma_start(contrib[k, :, q * tch:(q + 1) * tch, :],
                                      src[:, q * tch:(q + 1) * tch, :])
            nc.gpsimd.collective_compute(
                "AllGather", mybir.AluOpType.bypass,
                ins=[contrib[:]], outs=[gbuf[:]], replica_groups=GROUPS)

        def sel_other(ctx_pools, gbuf, tcols, c, ch):
            """Select other-direction k-blocks from gathered buffer, chunk c."""
            selsb, selps, hoth_pool = ctx_pools
            t0 = c * (ch // BL)
            t1 = t0 + ch // BL
            hoth = []
            for kb in range(4):
                s0 = selsb.tile([128, ch], F16, tag="s0")
                nc.sync.dma_start(s0[:], gbuf[0, kb, :, t0:t1, :])
                s1 = selsb.tile([128, ch], F16, tag="s1")
                nc.sync.dma_start(s1[:], gbuf[1, kb, :, t0:t1, :])
                p = selps.tile([128, ch], F32, tag="selps")
                nc.tensor.matmul(p[:], sel0_sb[:], s0[:], start=True, stop=False)
                nc.tensor.matmul(p[:], sel1_sb[:], s1[:], start=False, stop=True)
                ho = hoth_pool.tile([128, ch], F16, tag="hoth")
                nc.vector.tensor_copy(ho[:], p[:])
                hoth.append(ho)
            return hoth

        # ---------------- phase 1: xg0 ----------------
        with ExitStack() as ctx:
            xsb = ctx.enter_context(tc.tile_pool(name="xsb", bufs=1))
            x_sb = xsb.tile([128, NCOL], F16)
            nc.sync.dma_start(x_sb[:], xt[:])
            wp = ctx.enter_context(tc.tile_pool(name="wih0p", bufs=1))
            wih0_sb = wp.tile([128, 12 * 128], F16)
            nc.sync.dma_start(wih0_sb[:], wih0[:])
            if with_bias[0]:
                b0p = ctx.enter_context(tc.tile_pool(name="b0p", bufs=1))
                b0_sb = b0p.tile([128, 12], F32)
                nc.sync.dma_start(b0_sb[:], bias0[:])
                b0_ap = b0_sb[:]
            else:
                b0_ap = None
            xg_phase(ctx,
                     lambda m, k: wih0_sb[:, m * 128:(m + 1) * 128],
                     1,
                     lambda k, c: x_sb[:, c * CH:(c + 1) * CH],
                     xg0d, b0_ap, "x0")

        # ---------------- phase 2: L0 recurrence ----------------
        h0_scope = ExitStack()
        h0p = h0_scope.enter_context(tc.tile_pool(name="h0p", bufs=1))
        h0_hist = h0p.tile([128, 4 * T * BL], F16)
        with ExitStack() as ctx:
            wp = ctx.enter_context(tc.tile_pool(name="whh0p", bufs=1))
            whh0_sb = wp.tile([128, 48 * 128], F16)
            nc.sync.dma_start(whh0_sb[:], whh0[:])
            bz = ctx.enter_context(tc.tile_pool(name="bhn0p", bufs=1))
            if with_bhn[0]:
                bhn0_sb = bz.tile([128, 32], F32)
                nc.sync.dma_start(bhn0_sb[:], bhn0[:])
                bhn_ap = bhn0_sb[:]
            else:
                bhn_ap = None
            recurrence(ctx, xg0d, whh0_sb, h0_hist, bhn_ap, "r0")

        # ---------------- phase 3: exchange h0 ----------------
        exchange(h0_hist, 0, T, contrib0, g0)

        # ---------------- phase 4: xg1 ----------------
        with ExitStack() as ctx:
            wp = ctx.enter_context(tc.tile_pool(name="wih1p", bufs=1))
            wih1o_sb = wp.tile([128, 48 * 128], F16, tag="wo")
            nc.sync.dma_start(wih1o_sb[:], wih1_own[:])
            wih1x_sb = wp.tile([128, 48 * 128], F16, tag="wx")
            nc.sync.dma_start(wih1x_sb[:], wih1_oth[:])
            b1p = ctx.enter_context(tc.tile_pool(name="b1p", bufs=1))
            if with_bias[1]:
                b1_sb = b1p.tile([128, 12], F32)
                nc.sync.dma_start(b1_sb[:], bias1[:])
                b1_ap = b1_sb[:]
            else:
                b1_ap = None
            selsb = ctx.enter_context(tc.tile_pool(name="sl4", bufs=3))
            selps = ctx.enter_context(
                tc.tile_pool(name="slp4", bufs=2, space="PSUM"))
            hop = ctx.enter_context(tc.tile_pool(name="ho4", bufs=8))
            sb = ctx.enter_context(tc.tile_pool(name="x1sb", bufs=4))
            ps = ctx.enter_context(
                tc.tile_pool(name="x1ps", bufs=2, space="PSUM"))
            h0r_flat = h0_hist
            for c in range(NCH):
                hoth = sel_other((selsb, selps, hop), g0, T, c, CH)
                for m in range(12):
                    p = ps.tile([128, CH], F32, tag="x1p")
                    for k in range(4):
                        nc.tensor.matmul(
                            p[:],
                            wih1o_sb[:, (m * 4 + k) * 128:(m * 4 + k + 1) * 128],
                            h0r_flat[:, k * NCOL + c * CH: k * NCOL + (c + 1) * CH],
                            start=(k == 0), stop=False)
                    for k in range(4):
                        nc.tensor.matmul(
                            p[:],
                            wih1x_sb[:, (m * 4 + k) * 128:(m * 4 + k + 1) * 128],
                            hoth[k][:], start=False, stop=(k == 3))
                    o = sb.tile([128, CH], F16, tag="x1o")
                    if b1_ap is not None:
                        if m % 2 == 0:
                            nc.scalar.activation(o[:], p[:], AF.Identity,
                                                 bias=b1_ap[:, m:m + 1])
                        else:
                            nc.vector.tensor_scalar_add(o[:], p[:],
                                                        b1_ap[:, m:m + 1])
                    else:
                        if m % 2 == 0:
                            nc.scalar.copy(o[:], p[:])
                        else:
                            nc.vector.tensor_copy(o[:], p[:])
                    t0 = c * (CH // BL)
                    t1 = (c + 1) * (CH // BL)
                    nc.sync.dma_start(xg1d[:, m, t0:t1, :], o[:])
        h0_scope.close()

        # ---------------- phase 5: L1 recurrence ----------------
        h1_scope = ExitStack()
        h1p = h1_scope.enter_context(tc.tile_pool(name="h1p", bufs=1))
        h1_hist = h1p.tile([128, 4 * T * BL], F16)
        with ExitStack() as ctx:
            wp = ctx.enter_context(tc.tile_pool(name="whh1p", bufs=1))
            whh1_sb = wp.tile([128, 48 * 128], F16)
            nc.sync.dma_start(whh1_sb[:], whh1[:])
            bz = ctx.enter_context(tc.tile_pool(name="bhn1p", bufs=1))
            if with_bhn[1]:
                bhn1_sb = bz.tile([128, 32], F32)
                nc.sync.dma_start(bhn1_sb[:], bhn1[:])
                bhn_ap = bhn1_sb[:]
            else:
                bhn_ap = None
            recurrence(ctx, xg1d, whh1_sb, h1_hist, bhn_ap, "r1")

        # ---------------- phase 6: exchange h1 tail ----------------
        exchange(h1_hist, TH, TH, contrib1, g1)

        # ---------------- phase 7: attention + fc ----------------
        with ExitStack() as ctx:
            wp = ctx.enter_context(tc.tile_pool(name="awp", bufs=1))
            attno_sb = wp.tile([128, 32 * 128], F16, tag="ao")
            nc.sync.dma_start(attno_sb[:], attn_own[:])
            attnx_sb = wp.tile([128, 32 * 128], F16, tag="ax")
            nc.sync.dma_start(attnx_sb[:], attn_oth[:])
            fcw_sb = wp.tile([128, 8 * O], F16, tag="fw")
            nc.sync.dma_start(fcw_sb[:], fcw[:])
            ab_sb = wp.tile([128, 8], F32, tag="ab")
            if with_attn_bias:
                nc.sync.dma_start(ab_sb[:], attn_b[:])
            fb_sb = wp.tile([128, 1], F32, tag="fb")
            if with_fc_bias:
                nc.sync.dma_start(fb_sb[:], fc_b[:])

            selsb = ctx.enter_context(tc.tile_pool(name="sl7", bufs=3))
            selps = ctx.enter_context(
                tc.tile_pool(name="slp7", bufs=2, space="PSUM"))
            hop = ctx.enter_context(tc.tile_pool(name="ho7", bufs=8))
            sb = ctx.enter_context(tc.tile_pool(name="asb", bufs=4))
            aps = ctx.enter_context(
                tc.tile_pool(name="aps", bufs=2, space="PSUM"))
            fps = ctx.enter_context(
                tc.tile_pool(name="fps", bufs=2, space="PSUM"))
            for c in range(NCH2):
                hoth = sel_other((selsb, selps, hop), g1, TH, c, CH2)
                pf = fps.tile([O, CH2], F32, tag="fcp")
                for m in range(8):
                    p = aps.tile([128, CH2], F32, tag="ap")
                    for k in range(4):
                        nc.tensor.matmul(
                            p[:],
                            attno_sb[:, (m * 4 + k) * 128:(m * 4 + k + 1) * 128],
                            h1_hist[:, k * NCOL + c * CH2: k * NCOL + (c + 1) * CH2],
                            start=(k == 0), stop=False)
                    for k in range(4):
                        nc.tensor.matmul(
                            p[:],
                            attnx_sb[:, (m * 4 + k) * 128:(m * 4 + k + 1) * 128],
                            hoth[k][:], start=False, stop=(k == 3))
                    at = sb.tile([128, CH2], F32, tag="at")
                    if with_attn_bias:
                        nc.scalar.activation(at[:], p[:], AF.Tanh,
                                             bias=ab_sb[:, m:m + 1])
                    else:
                        nc.scalar.activation(at[:], p[:], AF.Tanh)
                    gt = sb.tile([128, CH2], F16, tag="gt")
                    if m < 4:
                        hloc = h1_hist[:, m * NCOL + c * CH2: m * NCOL + (c + 1) * CH2]
                    else:
                        hloc = hoth[m - 4][:]
                    nc.vector.tensor_mul(gt[:], at[:], hloc)
                    nc.tensor.matmul(pf[:], fcw_sb[:, m * O:(m + 1) * O], gt[:],
                                     start=(m == 0), stop=(m == 7))
                ot = sb.tile([O, CH2], F32, tag="ot")
                if with_fc_bias:
                    nc.scalar.activation(ot[:], pf[:], AF.Identity,
                                         bias=fb_sb[0:O, 0:1])
                else:
                    nc.scalar.copy(ot[:], pf[:])
                t0 = c * (CH2 // BL)
                t1 = (c + 1) * (CH2 // BL)
                nc.sync.dma_start(out_d[:, t0:t1, :], ot[:])
        h1_scope.close()

    nc.compile()
    return nc


# ----------------------------------------------------------------- host prep
def prep_core_inputs(inputs, c, T=T_FULL):
    d, g = c % 2, c // 2
    TH = T // 2
    f16 = lambda a: np.ascontiguousarray(a, dtype=np.float16)
    f32 = lambda a: np.ascontiguousarray(a, dtype=np.float32)

    x = np.asarray(inputs['x'])[g * BL:(g + 1) * BL, :T]      # [8, T, 128]
    if d == 1:
        x = x[:, ::-1]
    xt = f16(x.transpose(2, 1, 0).reshape(128, T * BL))

    w_hh0 = np.asarray(inputs['W_hh0'])[d]     # [1536, 512]
    w_hh1 = np.asarray(inputs['W_hh1'])[d]
    w_ih0 = np.asarray(inputs['W_ih0'])[d]     # [1536, 128]
    w_ih1 = np.asarray(inputs['W_ih1'])[d]     # [1536, 1024]
    b_ih0 = np.asarray(inputs['b_ih0'])[d]
    b_hh0 = np.asarray(inputs['b_hh0'])[d]
    b_ih1 = np.asarray(inputs['b_ih1'])[d]
    b_hh1 = np.asarray(inputs['b_hh1'])[d]
    attn_W = np.asarray(inputs['attn_W'])      # [1024, 1024]
    attn_bv = np.asarray(inputs['attn_b'])
    fc_W = np.asarray(inputs['fc_W'])          # [10, 1024]
    fc_bv = np.asarray(inputs['fc_b'])

    def whh_tiles(w):
        out = np.zeros((128, 48 * 128), np.float16)
        for j in range(12):
            rb = PERMROWS[j]
            for k in range(4):
                blk = w[rb * 128:(rb + 1) * 128, k * 128:(k + 1) * 128]
                out[:, (j * 4 + k) * 128:(j * 4 + k + 1) * 128] = \
                    blk.T.astype(np.float16)
        return out

    whh0 = whh_tiles(w_hh0)
    whh1 = whh_tiles(w_hh1)

    wih0 = np.zeros((128, 12 * 128), np.float16)
    for j in range(12):
        rb = PERMROWS[j]
        wih0[:, j * 128:(j + 1) * 128] = \
            w_ih0[rb * 128:(rb + 1) * 128, :].T.astype(np.float16)

    own_lo = 0 if d == 0 else 512
    oth_lo = 512 - own_lo

    def wih1_tiles(col_lo):
        out = np.zeros((128, 48 * 128), np.float16)
        for j in range(12):
            rb = PERMROWS[j]
            for k in range(4):
                blk = w_ih1[rb * 128:(rb + 1) * 128,
                            col_lo + k * 128: col_lo + (k + 1) * 128]
                out[:, (j * 4 + k) * 128:(j * 4 + k + 1) * 128] = \
                    blk.T.astype(np.float16)
        return out

    wih1_own = wih1_tiles(own_lo)
    wih1_oth = wih1_tiles(oth_lo)

    ident = np.eye(128, dtype=np.float16)
    zer = np.zeros((128, 128), np.float16)
    sel0 = ident if d == 1 else zer      # gathered rank0 = fwd core
    sel1 = ident if d == 0 else zer

    # attention: rows and cols in LOCAL order (own dims first)
    attn_local = np.concatenate(
        [attn_W[own_lo:own_lo + 512], attn_W[oth_lo:oth_lo + 512]], axis=0)

    def attn_tiles(col_lo):
        out = np.zeros((128, 32 * 128), np.float16)
        for m in range(8):
            for k in range(4):
                blk = attn_local[m * 128:(m + 1) * 128,
                                 col_lo + k * 128: col_lo + (k + 1) * 128]
                out[:, (m * 4 + k) * 128:(m * 4 + k + 1) * 128] = \
                    blk.T.astype(np.float16)
        return out

    attn_own = attn_tiles(own_lo)
    attn_oth = attn_tiles(oth_lo)

    fc_local = np.concatenate(
        [fc_W[:, own_lo:own_lo + 512], fc_W[:, oth_lo:oth_lo + 512]], axis=1)
    fcw = np.zeros((128, 8 * O), np.float16)
    for k in range(8):
        fcw[:, k * O:(k + 1) * O] = \
            fc_local[:, k * 128:(k + 1) * 128].T.astype(np.float16)

    # biases: fold b_ih + b_hh(r,z) into xg bias; n keeps b_ih only + bhn tile
    def gate_bias(b_ih, b_hh):
        v = b_ih.astype(np.float64).copy()
        v[:H] += b_hh[:H]              # r
        v[H:2 * H] += b_hh[H:2 * H]    # z
        bias = np.zeros((128, 12), np.float32)
        for j in range(12):
            rb = PERMROWS[j]
            bias[:, j] = v[rb * 128:(rb + 1) * 128]
        return bias

    bias0 = gate_bias(b_ih0, b_hh0)
    bias1 = gate_bias(b_ih1, b_hh1)
    bhn0 = np.zeros((128, 32), np.float32)
    bhn1 = np.zeros((128, 32), np.float32)
    for jj in range(4):
        bhn0[:, jj * 8:(jj + 1) * 8] = \
            b_hh0[2 * H + jj * 128: 2 * H + (jj + 1) * 128, None]
        bhn1[:, jj * 8:(jj + 1) * 8] = \
            b_hh1[2 * H + jj * 128: 2 * H + (jj + 1) * 128, None]

    attn_b_local = np.concatenate(
        [attn_bv[own_lo:own_lo + 512], attn_bv[oth_lo:oth_lo + 512]])
    attn_b = np.zeros((128, 8), np.float32)
    for m in range(8):
        attn_b[:, m] = attn_b_local[m * 128:(m + 1) * 128]
    fc_b = np.zeros((128, 1), np.float32)
    fc_b[:O, 0] = fc_bv

    return {
        "xt": xt, "whh0": whh0, "whh1": whh1, "wih0": wih0,
        "wih1_own": wih1_own, "wih1_oth": wih1_oth,
        "sel0": sel0, "sel1": sel1,
        "attn_own": attn_own, "attn_oth": attn_oth, "fcw": fcw,
        "bias0": f32(bias0), "bias1": f32(bias1),
        "bhn0": f32(bhn0), "bhn1": f32(bhn1),
        "attn_b": f32(attn_b), "fc_b": f32(fc_b),
    }


def flags_from_inputs(inputs):
    nz = lambda a: bool(np.any(np.asarray(a)))
    with_bhn = (nz(np.asarray(inputs['b_hh0'])[:, 2 * H:]),
                nz(np.asarray(inputs['b_hh1'])[:, 2 * H:]))
    with_bias = (nz(inputs['b_ih0']) or nz(np.asarray(inputs['b_hh0'])[:, :2 * H]),
                 nz(inputs['b_ih1']) or nz(np.asarray(inputs['b_hh1'])[:, :2 * H]))
    return dict(with_bhn=with_bhn, with_bias=with_bias,
                with_attn_bias=nz(inputs['attn_b']),
                with_fc_bias=nz(inputs['fc_b']))


_PROG_CACHE = {}


def _get_program(T, flags):
    key = (T, tuple(sorted((k, tuple(v) if isinstance(v, tuple) else v)
                           for k, v in flags.items())))
    if key not in _PROG_CACHE:
        _PROG_CACHE[key] = build_program(T=T, **flags)
    return _PROG_CACHE[key]


def run_cores(inputs, T=T_FULL, trace=False, **kw):
    flags = flags_from_inputs(inputs)
    nc = _get_program(T, flags)
    in_maps = [prep_core_inputs(inputs, c, T=T) for c in range(N_CORES)]
    res = run_bass_kernel_spmd(nc, in_maps, list(range(N_CORES)), trace=trace,
                               **kw)
    return res


def assemble_output(results, T=T_FULL):
    TH = T // 2
    out = np.zeros((B, T, O), np.float32)
    for c in range(N_CORES):
        d, g = c % 2, c // 2
        r = results[c]["out"].transpose(2, 1, 0)   # [O,TH,BL] -> [BL,TH,O]
        if d == 0:
            out[g * BL:(g + 1) * BL, :TH] = r
        else:
            out[g * BL:(g + 1) * BL, TH:] = r[:, ::-1, :]
    return out


def kernel(**inputs) -> np.ndarray:
    res = run_cores(inputs, T=T_FULL)
    return assemble_output(res.results, T=T_FULL)


if __name__ == "__main__":
    pass


# revision 19
# speedup vs baseline: 11.1896x; 1.7840x over previous
"""Trainium2 Bass kernel for nn_BiGRUWithAttention.

Model: x -> BiGRU(128->512) -> BiGRU(1024->512) -> attn=tanh(h@Wa.T+ba) ->
       gated=attn*h -> out = gated@Wf.T+bf   (B=32, T=1024, out 10)

Sharding: 8 cores = 4 batch groups (8 samples each) x 2 directions.
Core c: dir d=c%2 (0=fwd, 1=bwd), group g=c//2. Replica pairs [[0,1],..].
The SPMD program is direction-agnostic: every core runs a *forward*
recurrence over its local time order tau (bwd cores get time-reversed
inputs prepared on the host). The fwd/bwd exchange between layers is an
AllGather of time-reversed hidden states plus a matmul-based selection
(host-supplied identity/zero matrices) so no core ever branches on its
rank.

Layouts (per core, everything "dims on partitions"):
  h_hist sbuf [128, 4*T*8] fp16   : col = k_block*T*8 + tau*8 + b
  gate psum  [128, 32] per gate   : M-tile j -> cols, block order r,n,z
  xg dram    [128, 12, T, 8] fp16 : precomputed input gates (bias folded)
"""
import sys, os
sys.path.insert(0, '/opt/trn_rl_repo')

import numpy as np
from contextlib import ExitStack

import concourse.bass as bass
import concourse.bacc as bacc
import concourse.tile as tile
from concourse import mybir
from concourse.bass_utils import run_bass_kernel_spmd

F16 = mybir.dt.float16
F32 = mybir.dt.float32
AF = mybir.ActivationFunctionType

N_CORES = 8
B, T_FULL, I_IN, H, O = 32, 1024, 128, 512, 10
G = 3 * H            # 1536 gate dims = 12 tiles of 128
BL = 8               # batch per core
# psum M-tile j -> row-block of W_hh/W_ih (gates stacked r,z,n in weights;
# psum layout r(j 0-3), n(j 4-7), z(j 8-11))
PERMROWS = [0, 1, 2, 3, 8, 9, 10, 11, 4, 5, 6, 7]
GROUPS = [[0, 1], [2, 3], [4, 5], [6, 7]]


# ----------------------------------------------------------------- program
def build_program(T=T_FULL, with_bhn=(False, False), with_bias=(False, False),
                  with_attn_bias=False, with_fc_bias=False):
    TH = T // 2
    NCOL = T * BL               # columns of the full sequence
    CH = min(512, NCOL)         # chunk width for big GEMM phases
    NCH = NCOL // CH
    NCOL2 = TH * BL             # attention token columns per core
    CH2 = min(512, NCOL2)
    NCH2 = NCOL2 // CH2
    XB = min(8, T)              # recurrence xg prefetch batch (steps)

    nc = bacc.Bacc("TRN2", target_bir_lowering=False, debug=False,
                   num_devices=N_CORES)

    def din(name, shape, dt=F16):
        return nc.dram_tensor(name, shape, dt, kind="ExternalInput").ap()

    xt = din("xt", [128, NCOL])                       # x.T (I on partitions)
    whh0 = din("whh0", [128, 48 * 128])
    whh1 = din("whh1", [128, 48 * 128])
    wih0 = din("wih0", [128, 12 * 128])
    wih1_own = din("wih1_own", [128, 48 * 128])
    wih1_oth = din("wih1_oth", [128, 48 * 128])
    sel0 = din("sel0", [128, 128])
    sel1 = din("sel1", [128, 128])
    attn_own = din("attn_own", [128, 32 * 128])
    attn_oth = din("attn_oth", [128, 32 * 128])
    fcw = din("fcw", [128, 8 * O])
    bias0 = din("bias0", [128, 12], F32)
    bias1 = din("bias1", [128, 12], F32)
    bhn0 = din("bhn0", [128, 32], F32)
    bhn1 = din("bhn1", [128, 32], F32)
    attn_b = din("attn_b", [128, 8], F32)
    fc_b = din("fc_b", [128, 1], F32)

    out_d = nc.dram_tensor("out", [O, TH, BL], F32, kind="ExternalOutput").ap()

    xg0d = nc.dram_tensor("xg0d", [128, 12, T * BL], F16).ap()
    xg1d = nc.dram_tensor("xg1d", [128, 12, T * BL], F16).ap()
    contrib0 = nc.dram_tensor("contrib0", [4, 128, T, BL], F16).ap()
    g0 = nc.dram_tensor("g0", [2, 4, 128, T, BL], F16).ap()
    contrib1 = nc.dram_tensor("contrib1", [4, 128, TH, BL], F16).ap()
    g1 = nc.dram_tensor("g1", [2, 4, 128, TH, BL], F16).ap()

    with ExitStack() as top:
        tc = top.enter_context(tile.TileContext(nc))

        const = top.enter_context(tc.tile_pool(name="const", bufs=1))
        # constants that live for the whole kernel
        sel0_sb = const.tile([128, 128], F16)
        sel1_sb = const.tile([128, 128], F16)
        nc.sync.dma_start(sel0_sb[:], sel0[:])
        nc.sync.dma_start(sel1_sb[:], sel1[:])

        # ---------------- phase helpers ----------------
        def xg_phase(ctx, wih_tiles, nk, rhs_of_k, xgd, bias_ap, namep):
            """xg[m] = sum_k W[m,k] @ rhs_k  (+bias) -> xgd dram (fp16)."""
            sb = ctx.enter_context(tc.tile_pool(name=namep + "sb", bufs=4))
            ps = ctx.enter_context(
                tc.tile_pool(name=namep + "ps", bufs=2, space="PSUM"))
            for c in range(NCH):
                for m in range(12):
                    p = ps.tile([128, CH], F32, tag="xgps")
                    for k in range(nk):
                        nc.tensor.matmul(
                            p[:], wih_tiles(m, k), rhs_of_k(k, c),
                            start=(k == 0), stop=(k == nk - 1))
                    o = sb.tile([128, CH], F16, tag="xgsb")
                    if bias_ap is not None:
                        if m % 2 == 0:
                            nc.scalar.activation(o[:], p[:], AF.Identity,
                                                 bias=bias_ap[:, m:m + 1])
                        else:
                            nc.vector.tensor_scalar_add(o[:], p[:],
                                                        bias_ap[:, m:m + 1])
                    else:
                        if m % 2 == 0:
                            nc.scalar.copy(o[:], p[:])
                        else:
                            nc.vector.tensor_copy(o[:], p[:])
                    nc.sync.dma_start(xgd[:, m, c * CH:(c + 1) * CH], o[:])

        def recurrence(ctx, xgd, whh_sb, h_hist, bhn_ap, namep):
            hr = h_hist[:].rearrange("p (k t b) -> p k t b", k=4, t=T)
            xgp = ctx.enter_context(tc.tile_pool(name=namep + "xg", bufs=3))
            tmp = ctx.enter_context(tc.tile_pool(name=namep + "tmp", bufs=3))
            hp = ctx.enter_context(tc.tile_pool(name=namep + "h32", bufs=2))
            psr = ctx.enter_context(
                tc.tile_pool(name=namep + "psr", bufs=2, space="PSUM"))
            psn = ctx.enter_context(
                tc.tile_pool(name=namep + "psn", bufs=2, space="PSUM"))
            psz = ctx.enter_context(
                tc.tile_pool(name=namep + "psz", bufs=2, space="PSUM"))
            zpool = ctx.enter_context(tc.tile_pool(name=namep + "z", bufs=1))

            zrhs = zpool.tile([128, 32], F16)
            nc.vector.memset(zrhs[:], 0.0)
            h32_prev = zpool.tile([128, 32], F32)
            nc.vector.memset(h32_prev[:], 0.0)

            xgc = None
            for t in range(T):
                if t % XB == 0:
                    xgc = xgp.tile([128, 12 * XB * BL], F16, tag="xgc")
                    nc.sync.dma_start(
                        xgc[:], xgd[:, :, t * BL:(t + XB) * BL])
                xv = xgc[:].rearrange("p (m t b) -> p m t b", m=12, t=XB)
                tl = t % XB

                def rhs(k):
                    if t == 0:
                        return zrhs[:, k * 8:(k + 1) * 8]
                    return hr[:, k, t - 1, :]

                def mm(ps_t, j):
                    jo = j % 4
                    for k in range(4):
                        nc.tensor.matmul(
                            ps_t[:, jo * 8:(jo + 1) * 8],
                            whh_sb[:, (j * 4 + k) * 128:(j * 4 + k + 1) * 128],
                            rhs(k), start=(k == 0), stop=(k == 3))

                pr = psr.tile([128, 32], F32, tag="pr")
                for j in range(0, 4):
                    mm(pr, j)
                tr = tmp.tile([128, 32], F32, tag="tr")
                nc.vector.tensor_add(tr[:].rearrange("p (m b) -> p m b", m=4),
                                     pr[:].rearrange("p (m b) -> p m b", m=4),
                                     xv[:, 0:4, tl, :])
                rg = tmp.tile([128, 32], F32, tag="rg")
                nc.scalar.activation(rg[:], tr[:], AF.Sigmoid)

                pn = psn.tile([128, 32], F32, tag="pn")
                for j in range(4, 8):
                    mm(pn, j)
                if bhn_ap is not None:
                    t1_ = tmp.tile([128, 32], F32, tag="t1")
                    nc.vector.tensor_add(t1_[:], pn[:], bhn_ap)
                    nsrc = t1_
                else:
                    nsrc = pn
                t2 = tmp.tile([128, 32], F32, tag="t2")
                nc.vector.tensor_mul(t2[:], nsrc[:], rg[:])
                t3 = tmp.tile([128, 32], F32, tag="t3")
                nc.vector.tensor_add(t3[:].rearrange("p (m b) -> p m b", m=4),
                                     t2[:].rearrange("p (m b) -> p m b", m=4),
                                     xv[:, 4:8, tl, :])
                ng = tmp.tile([128, 32], F32, tag="ng")
                nc.scalar.activation(ng[:], t3[:], AF.Tanh)

                pz = psz.tile([128, 32], F32, tag="pz")
                for j in range(8, 12):
                    mm(pz, j)
                tz = tmp.tile([128, 32], F32, tag="tz")
                nc.vector.tensor_add(tz[:].rearrange("p (m b) -> p m b", m=4),
                                     pz[:].rearrange("p (m b) -> p m b", m=4),
                                     xv[:, 8:12, tl, :])
                zg = tmp.tile([128, 32], F32, tag="zg")
                nc.scalar.activation(zg[:], tz[:], AF.Sigmoid)

                dd = tmp.tile([128, 32], F32, tag="dd")
                nc.vector.tensor_sub(dd[:], h32_prev[:], ng[:])
                ee = tmp.tile([128, 32], F32, tag="ee")
                nc.vector.tensor_mul(ee[:], zg[:], dd[:])
                h32 = hp.tile([128, 32], F32, tag="h32")
                nc.vector.tensor_add(h32[:], ng[:], ee[:])
                nc.vector.tensor_copy(
                    hr[:, :, t, :],
                    h32[:].rearrange("p (k b) -> p k b", k=4))
                h32_prev = h32

        def exchange(h_hist, t_lo, t_cnt, contrib, gbuf):
            """contrib[i] = h_hist[t_lo + i] (forward); AllGather -> gbuf.
            Receivers un-reverse inside the select matmul's moving AP."""
            hr = h_hist[:].rearrange("p (k c) -> p k c", k=4)
            tch = 256 * BL
            ccnt = t_cnt * BL
            cfl = contrib[:].rearrange("k p t b -> k p (t b)")
            for k in range(4):
                src = hr[:, k, t_lo * BL: (t_lo + t_cnt) * BL]
                for q in range((ccnt + tch - 1) // tch):
                    s = slice(q * tch, min((q + 1) * tch, ccnt))
                    nc.sync.dma_start(cfl[k, :, s], src[:, s])
            nc.gpsimd.collective_compute(
                "AllGather", mybir.AluOpType.bypass,
                ins=[contrib[:]], outs=[gbuf[:]], replica_groups=GROUPS)

        def sel_other(ctx_pools, gbuf, nch, c, ch):
            """Select other-dir k-blocks for target chunk c (local time order).

            The gathered buffer holds the donor's hidden states in donor time
            order; local order is the full reversal, so source chunk is the
            mirrored one, read with a reversed-tau moving AP."""
            selsb, selps, hoth_pool = ctx_pools
            cs = nch - 1 - c                     # mirrored source chunk
            t0 = cs * (ch // BL)
            t1 = t0 + ch // BL
            hoth = []
            for kb in range(4):
                s0 = selsb.tile([128, ch], F16, tag="s0")
                nc.sync.dma_start(
                    s0[:], gbuf[0, kb].rearrange("p t b -> p (t b)")
                    [:, t0 * BL:t1 * BL])
                s1 = selsb.tile([128, ch], F16, tag="s1")
                nc.sync.dma_start(
                    s1[:], gbuf[1, kb].rearrange("p t b -> p (t b)")
                    [:, t0 * BL:t1 * BL])
                p = selps.tile([128, ch], F32, tag="selps")
                r0 = s0[:].rearrange("p (t b) -> p t b", b=BL)[:, ::-1, :]
                r1 = s1[:].rearrange("p (t b) -> p t b", b=BL)[:, ::-1, :]
                nc.tensor.matmul(p[:], sel0_sb[:], r0, start=True, stop=False)
                nc.tensor.matmul(p[:], sel1_sb[:], r1, start=False, stop=True)
                ho = hoth_pool.tile([128, ch], F16, tag="hoth")
                nc.vector.tensor_copy(ho[:], p[:])
                hoth.append(ho)
            return hoth

        # ---------------- phase 1: xg0 ----------------
        with ExitStack() as ctx:
            xsb = ctx.enter_context(tc.tile_pool(name="xsb", bufs=1))
            x_sb = xsb.tile([128, NCOL], F16)
            nc.sync.dma_start(x_sb[:], xt[:])
            wp = ctx.enter_context(tc.tile_pool(name="wih0p", bufs=1))
            wih0_sb = wp.tile([128, 12 * 128], F16)
            nc.sync.dma_start(wih0_sb[:], wih0[:])
            if with_bias[0]:
                b0p = ctx.enter_context(tc.tile_pool(name="b0p", bufs=1))
                b0_sb = b0p.tile([128, 12], F32)
                nc.sync.dma_start(b0_sb[:], bias0[:])
                b0_ap = b0_sb[:]
            else:
                b0_ap = None
            xg_phase(ctx,
                     lambda m, k: wih0_sb[:, m * 128:(m + 1) * 128],
                     1,
                     lambda k, c: x_sb[:, c * CH:(c + 1) * CH],
                     xg0d, b0_ap, "x0")

        # ---------------- phase 2: L0 recurrence ----------------
        h0_scope = ExitStack()
        h0p = h0_scope.enter_context(tc.tile_pool(name="h0p", bufs=1))
        h0_hist = h0p.tile([128, 4 * T * BL], F16)
        with ExitStack() as ctx:
            wp = ctx.enter_context(tc.tile_pool(name="whh0p", bufs=1))
            whh0_sb = wp.tile([128, 48 * 128], F16)
            nc.sync.dma_start(whh0_sb[:], whh0[:])
            bz = ctx.enter_context(tc.tile_pool(name="bhn0p", bufs=1))
            if with_bhn[0]:
                bhn0_sb = bz.tile([128, 32], F32)
                nc.sync.dma_start(bhn0_sb[:], bhn0[:])
                bhn_ap = bhn0_sb[:]
            else:
                bhn_ap = None
            recurrence(ctx, xg0d, whh0_sb, h0_hist, bhn_ap, "r0")

        # ---------------- phase 3: exchange h0 ----------------
        exchange(h0_hist, 0, T, contrib0, g0)

        # ---------------- phase 4: xg1 ----------------
        with ExitStack() as ctx:
            wp = ctx.enter_context(tc.tile_pool(name="wih1p", bufs=1))
            wih1o_sb = wp.tile([128, 48 * 128], F16, tag="wo")
            nc.sync.dma_start(wih1o_sb[:], wih1_own[:])
            wih1x_sb = wp.tile([128, 48 * 128], F16, tag="wx")
            nc.sync.dma_start(wih1x_sb[:], wih1_oth[:])
            b1p = ctx.enter_context(tc.tile_pool(name="b1p", bufs=1))
            if with_bias[1]:
                b1_sb = b1p.tile([128, 12], F32)
                nc.sync.dma_start(b1_sb[:], bias1[:])
                b1_ap = b1_sb[:]
            else:
                b1_ap = None
            selsb = ctx.enter_context(tc.tile_pool(name="sl4", bufs=3))
            selps = ctx.enter_context(
                tc.tile_pool(name="slp4", bufs=2, space="PSUM"))
            hop = ctx.enter_context(tc.tile_pool(name="ho4", bufs=8))
            sb = ctx.enter_context(tc.tile_pool(name="x1sb", bufs=4))
            ps = ctx.enter_context(
                tc.tile_pool(name="x1ps", bufs=2, space="PSUM"))
            h0r_flat = h0_hist
            for c in range(NCH):
                hoth = sel_other((selsb, selps, hop), g0, NCH, c, CH)
                for m in range(12):
                    p = ps.tile([128, CH], F32, tag="x1p")
                    for k in range(4):
                        nc.tensor.matmul(
                            p[:],
                            wih1o_sb[:, (m * 4 + k) * 128:(m * 4 + k + 1) * 128],
                            h0r_flat[:, k * NCOL + c * CH: k * NCOL + (c + 1) * CH],
                            start=(k == 0), stop=False)
                    for k in range(4):
                        nc.tensor.matmul(
                            p[:],
                            wih1x_sb[:, (m * 4 + k) * 128:(m * 4 + k + 1) * 128],
                            hoth[k][:], start=False, stop=(k == 3))
                    o = sb.tile([128, CH], F16, tag="x1o")
                    if b1_ap is not None:
                        if m % 2 == 0:
                            nc.scalar.activation(o[:], p[:], AF.Identity,
                                                 bias=b1_ap[:, m:m + 1])
                        else:
                            nc.vector.tensor_scalar_add(o[:], p[:],
                                                        b1_ap[:, m:m + 1])
                    else:
                        if m % 2 == 0:
                            nc.scalar.copy(o[:], p[:])
                        else:
                            nc.vector.tensor_copy(o[:], p[:])
                    nc.sync.dma_start(xg1d[:, m, c * CH:(c + 1) * CH], o[:])
        h0_scope.close()

        # ---------------- phase 5: L1 recurrence ----------------
        h1_scope = ExitStack()
        h1p = h1_scope.enter_context(tc.tile_pool(name="h1p", bufs=1))
        h1_hist = h1p.tile([128, 4 * T * BL], F16)
        with ExitStack() as ctx:
            wp = ctx.enter_context(tc.tile_pool(name="whh1p", bufs=1))
            whh1_sb = wp.tile([128, 48 * 128], F16)
            nc.sync.dma_start(whh1_sb[:], whh1[:])
            bz = ctx.enter_context(tc.tile_pool(name="bhn1p", bufs=1))
            if with_bhn[1]:
                bhn1_sb = bz.tile([128, 32], F32)
                nc.sync.dma_start(bhn1_sb[:], bhn1[:])
                bhn_ap = bhn1_sb[:]
            else:
                bhn_ap = None
            recurrence(ctx, xg1d, whh1_sb, h1_hist, bhn_ap, "r1")

        # ---------------- phase 6: exchange h1 tail ----------------
        exchange(h1_hist, TH, TH, contrib1, g1)

        # ---------------- phase 7: attention + fc ----------------
        with ExitStack() as ctx:
            wp = ctx.enter_context(tc.tile_pool(name="awp", bufs=1))
            attno_sb = wp.tile([128, 32 * 128], F16, tag="ao")
            nc.sync.dma_start(attno_sb[:], attn_own[:])
            attnx_sb = wp.tile([128, 32 * 128], F16, tag="ax")
            nc.sync.dma_start(attnx_sb[:], attn_oth[:])
            fcw_sb = wp.tile([128, 8 * O], F16, tag="fw")
            nc.sync.dma_start(fcw_sb[:], fcw[:])
            ab_sb = wp.tile([128, 8], F32, tag="ab")
            if with_attn_bias:
                nc.sync.dma_start(ab_sb[:], attn_b[:])
            fb_sb = wp.tile([128, 1], F32, tag="fb")
            if with_fc_bias:
                nc.sync.dma_start(fb_sb[:], fc_b[:])

            selsb = ctx.enter_context(tc.tile_pool(name="sl7", bufs=3))
            selps = ctx.enter_context(
                tc.tile_pool(name="slp7", bufs=2, space="PSUM"))
            hop = ctx.enter_context(tc.tile_pool(name="ho7", bufs=8))
            sb = ctx.enter_context(tc.tile_pool(name="asb", bufs=4))
            aps = ctx.enter_context(
                tc.tile_pool(name="aps", bufs=2, space="PSUM"))
            fps = ctx.enter_context(
                tc.tile_pool(name="fps", bufs=2, space="PSUM"))
            for c in range(NCH2):
                hoth = sel_other((selsb, selps, hop), g1, NCH2, c, CH2)
                pf = fps.tile([O, CH2], F32, tag="fcp")
                for m in range(8):
                    p = aps.tile([128, CH2], F32, tag="ap")
                    for k in range(4):
                        nc.tensor.matmul(
                            p[:],
                            attno_sb[:, (m * 4 + k) * 128:(m * 4 + k + 1) * 128],
                            h1_hist[:, k * NCOL + c * CH2: k * NCOL + (c + 1) * CH2],
                            start=(k == 0), stop=False)
                    for k in range(4):
                        nc.tensor.matmul(
                            p[:],
                            attnx_sb[:, (m * 4 + k) * 128:(m * 4 + k + 1) * 128],
                            hoth[k][:], start=False, stop=(k == 3))
                    at = sb.tile([128, CH2], F32, tag="at")
                    if with_attn_bias:
                        nc.scalar.activation(at[:], p[:], AF.Tanh,
                                             bias=ab_sb[:, m:m + 1])
                    else:
                        nc.scalar.activation(at[:], p[:], AF.Tanh)
                    gt = sb.tile([128, CH2], F16, tag="gt")
                    if m < 4:
                        hloc = h1_hist[:, m * NCOL + c * CH2: m * NCOL + (c + 1) * CH2]
                    else:
                        hloc = hoth[m - 4][:]
                    nc.vector.tensor_mul(gt[:], at[:], hloc)
                    nc.tensor.matmul(pf[:], fcw_sb[:, m * O:(m + 1) * O], gt[:],
                                     start=(m == 0), stop=(m == 7))
                ot = sb.tile([O, CH2], F32, tag="ot")
                if with_fc_bias:
                    nc.scalar.activation(ot[:], pf[:], AF.Identity,
                                         bias=fb_sb[0:O, 0:1])
                else:
                    nc.scalar.copy(ot[:], pf[:])
                t0 = c * (CH2 // BL)
                t1 = (c + 1) * (CH2 // BL)
                nc.sync.dma_start(out_d[:, t0:t1, :], ot[:])
        h1_scope.close()

    nc.compile()
    return nc


# ----------------------------------------------------------------- host prep
def prep_core_inputs(inputs, c, T=T_FULL):
    d, g = c % 2, c // 2
    TH = T // 2
    f16 = lambda a: np.ascontiguousarray(a, dtype=np.float16)
    f32 = lambda a: np.ascontiguousarray(a, dtype=np.float32)

    x = np.asarray(inputs['x'])[g * BL:(g + 1) * BL, :T]      # [8, T, 128]
    if d == 1:
        x = x[:, ::-1]
    xt = f16(x.transpose(2, 1, 0).reshape(128, T * BL))

    w_hh0 = np.asarray(inputs['W_hh0'])[d]     # [1536, 512]
    w_hh1 = np.asarray(inputs['W_hh1'])[d]
    w_ih0 = np.asarray(inputs['W_ih0'])[d]     # [1536, 128]
    w_ih1 = np.asarray(inputs['W_ih1'])[d]     # [1536, 1024]
    b_ih0 = np.asarray(inputs['b_ih0'])[d]
    b_hh0 = np.asarray(inputs['b_hh0'])[d]
    b_ih1 = np.asarray(inputs['b_ih1'])[d]
    b_hh1 = np.asarray(inputs['b_hh1'])[d]
    attn_W = np.asarray(inputs['attn_W'])      # [1024, 1024]
    attn_bv = np.asarray(inputs['attn_b'])
    fc_W = np.asarray(inputs['fc_W'])          # [10, 1024]
    fc_bv = np.asarray(inputs['fc_b'])

    def whh_tiles(w):
        out = np.zeros((128, 48 * 128), np.float16)
        for j in range(12):
            rb = PERMROWS[j]
            for k in range(4):
                blk = w[rb * 128:(rb + 1) * 128, k * 128:(k + 1) * 128]
                out[:, (j * 4 + k) * 128:(j * 4 + k + 1) * 128] = \
                    blk.T.astype(np.float16)
        return out

    whh0 = whh_tiles(w_hh0)
    whh1 = whh_tiles(w_hh1)

    wih0 = np.zeros((128, 12 * 128), np.float16)
    for j in range(12):
        rb = PERMROWS[j]
        wih0[:, j * 128:(j + 1) * 128] = \
            w_ih0[rb * 128:(rb + 1) * 128, :].T.astype(np.float16)

    own_lo = 0 if d == 0 else 512
    oth_lo = 512 - own_lo

    def wih1_tiles(col_lo):
        out = np.zeros((128, 48 * 128), np.float16)
        for j in range(12):
            rb = PERMROWS[j]
            for k in range(4):
                blk = w_ih1[rb * 128:(rb + 1) * 128,
                            col_lo + k * 128: col_lo + (k + 1) * 128]
                out[:, (j * 4 + k) * 128:(j * 4 + k + 1) * 128] = \
                    blk.T.astype(np.float16)
        return out

    wih1_own = wih1_tiles(own_lo)
    wih1_oth = wih1_tiles(oth_lo)

    ident = np.eye(128, dtype=np.float16)
    zer = np.zeros((128, 128), np.float16)
    sel0 = ident if d == 1 else zer      # gathered rank0 = fwd core
    sel1 = ident if d == 0 else zer

    # attention: rows and cols in LOCAL order (own dims first)
    attn_local = np.concatenate(
        [attn_W[own_lo:own_lo + 512], attn_W[oth_lo:oth_lo + 512]], axis=0)

    def attn_tiles(col_lo):
        out = np.zeros((128, 32 * 128), np.float16)
        for m in range(8):
            for k in range(4):
                blk = attn_local[m * 128:(m + 1) * 128,
                                 col_lo + k * 128: col_lo + (k + 1) * 128]
                out[:, (m * 4 + k) * 128:(m * 4 + k + 1) * 128] = \
                    blk.T.astype(np.float16)
        return out

    attn_own = attn_tiles(own_lo)
    attn_oth = attn_tiles(oth_lo)

    fc_local = np.concatenate(
        [fc_W[:, own_lo:own_lo + 512], fc_W[:, oth_lo:oth_lo + 512]], axis=1)
    fcw = np.zeros((128, 8 * O), np.float16)
    for k in range(8):
        fcw[:, k * O:(k + 1) * O] = \
            fc_local[:, k * 128:(k + 1) * 128].T.astype(np.float16)

    # biases: fold b_ih + b_hh(r,z) into xg bias; n keeps b_ih only + bhn tile
    def gate_bias(b_ih, b_hh):
        v = b_ih.astype(np.float64).copy()
        v[:H] += b_hh[:H]              # r
        v[H:2 * H] += b_hh[H:2 * H]    # z
        bias = np.zeros((128, 12), np.float32)
        for j in range(12):
            rb = PERMROWS[j]
            bias[:, j] = v[rb * 128:(rb + 1) * 128]
        return bias

    bias0 = gate_bias(b_ih0, b_hh0)
    bias1 = gate_bias(b_ih1, b_hh1)
    bhn0 = np.zeros((128, 32), np.float32)
    bhn1 = np.zeros((128, 32), np.float32)
    for jj in range(4):
        bhn0[:, jj * 8:(jj + 1) * 8] = \
            b_hh0[2 * H + jj * 128: 2 * H + (jj + 1) * 128, None]
        bhn1[:, jj * 8:(jj + 1) * 8] = \
            b_hh1[2 * H + jj * 128: 2 * H + (jj + 1) * 128, None]

    attn_b_local = np.concatenate(
        [attn_bv[own_lo:own_lo + 512], attn_bv[oth_lo:oth_lo + 512]])
    attn_b = np.zeros((128, 8), np.float32)
    for m in range(8):
        attn_b[:, m] = attn_b_local[m * 128:(m + 1) * 128]
    fc_b = np.zeros((128, 1), np.float32)
    fc_b[:O, 0] = fc_bv

    return {
        "xt": xt, "whh0": whh0, "whh1": whh1, "wih0": wih0,
        "wih1_own": wih1_own, "wih1_oth": wih1_oth,
        "sel0": sel0, "sel1": sel1,
        "attn_own": attn_own, "attn_oth": attn_oth, "fcw": fcw,
        "bias0": f32(bias0), "bias1": f32(bias1),
        "bhn0": f32(bhn0), "bhn1": f32(bhn1),
        "attn_b": f32(attn_b), "fc_b": f32(fc_b),
    }


def flags_from_inputs(inputs):
    nz = lambda a: bool(np.any(np.asarray(a)))
    with_bhn = (nz(np.asarray(inputs['b_hh0'])[:, 2 * H:]),
                nz(np.asarray(inputs['b_hh1'])[:, 2 * H:]))
    with_bias = (nz(inputs['b_ih0']) or nz(np.asarray(inputs['b_hh0'])[:, :2 * H]),
                 nz(inputs['b_ih1']) or nz(np.asarray(inputs['b_hh1'])[:, :2 * H]))
    return dict(with_bhn=with_bhn, with_bias=with_bias,
                with_attn_bias=nz(inputs['attn_b']),
                with_fc_bias=nz(inputs['fc_b']))


_PROG_CACHE = {}


def _get_program(T, flags):
    key = (T, tuple(sorted((k, tuple(v) if isinstance(v, tuple) else v)
                           for k, v in flags.items())))
    if key not in _PROG_CACHE:
        _PROG_CACHE[key] = build_program(T=T, **flags)
    return _PROG_CACHE[key]


def run_cores(inputs, T=T_FULL, trace=False, **kw):
    flags = flags_from_inputs(inputs)
    nc = _get_program(T, flags)
    in_maps = [prep_core_inputs(inputs, c, T=T) for c in range(N_CORES)]
    res = run_bass_kernel_spmd(nc, in_maps, list(range(N_CORES)), trace=trace,
                               **kw)
    return res


def assemble_output(results, T=T_FULL):
    TH = T // 2
    out = np.zeros((B, T, O), np.float32)
    for c in range(N_CORES):
        d, g = c % 2, c // 2
        r = results[c]["out"].transpose(2, 1, 0)   # [O,TH,BL] -> [BL,TH,O]
        if d == 0:
            out[g * BL:(g + 1) * BL, :TH] = r
        else:
            out[g * BL:(g + 1) * BL, TH:] = r[:, ::-1, :]
    return out


def kernel(**inputs) -> np.ndarray:
    res = run_cores(inputs, T=T_FULL)
    return assemble_output(res.results, T=T_FULL)


if __name__ == "__main__":
    pass


# revision 24
# speedup vs baseline: 11.8943x; 1.0630x over previous
"""Trainium2 Bass kernel for nn_BiGRUWithAttention.

Model: x -> BiGRU(128->512) -> BiGRU(1024->512) -> attn=tanh(h@Wa.T+ba) ->
       gated=attn*h -> out = gated@Wf.T+bf   (B=32, T=1024, out 10)

Sharding: 8 cores = 4 batch groups (8 samples each) x 2 directions.
Core c: dir d=c%2 (0=fwd, 1=bwd), group g=c//2. Replica pairs [[0,1],..].
The SPMD program is direction-agnostic: every core runs a *forward*
recurrence over its local time order tau (bwd cores get time-reversed
inputs prepared on the host). The fwd/bwd exchange between layers is an
AllGather of time-reversed hidden states plus a matmul-based selection
(host-supplied identity/zero matrices) so no core ever branches on its
rank.

Layouts (per core, everything "dims on partitions"):
  h_hist sbuf [128, 4*T*8] fp16   : col = k_block*T*8 + tau*8 + b
  gate psum  [128, 32] per gate   : M-tile j -> cols, block order r,n,z
  xg dram    [128, 12, T, 8] fp16 : precomputed input gates (bias folded)
"""
import sys, os
sys.path.insert(0, '/opt/trn_rl_repo')

import numpy as np
from contextlib import ExitStack

import concourse.bass as bass
import concourse.bacc as bacc
import concourse.tile as tile
from concourse import mybir
from concourse.bass_utils import run_bass_kernel_spmd

F16 = mybir.dt.float16
F32 = mybir.dt.float32
AF = mybir.ActivationFunctionType

N_CORES = 8
B, T_FULL, I_IN, H, O = 32, 1024, 128, 512, 10
G = 3 * H            # 1536 gate dims = 12 tiles of 128
BL = 8               # batch per core
# psum M-tile j -> row-block of W_hh/W_ih (gates stacked r,z,n in weights;
# psum layout r(j 0-3), n(j 4-7), z(j 8-11))
PERMROWS = [0, 1, 2, 3, 8, 9, 10, 11, 4, 5, 6, 7]
GROUPS = [[0, 1], [2, 3], [4, 5], [6, 7]]


# ----------------------------------------------------------------- program
def build_program(T=T_FULL, with_bhn=(False, False), with_bias=(False, False),
                  with_attn_bias=False, with_fc_bias=False):
    TH = T // 2
    NCOL = T * BL               # columns of the full sequence
    CH = min(512, NCOL)         # chunk width for big GEMM phases
    NCH = NCOL // CH
    NCOL2 = TH * BL             # attention token columns per core
    CH2 = min(512, NCOL2)
    NCH2 = NCOL2 // CH2
    XB = min(8, T)              # recurrence xg prefetch batch (steps)

    nc = bacc.Bacc("TRN2", target_bir_lowering=False, debug=False,
                   num_devices=N_CORES)

    def din(name, shape, dt=F16):
        return nc.dram_tensor(name, shape, dt, kind="ExternalInput").ap()

    xt = din("xt", [128, NCOL])                       # x.T (I on partitions)
    whh0 = din("whh0", [128, 48 * 128])
    whh1 = din("whh1", [128, 48 * 128])
    wih0 = din("wih0", [128, 12 * 128])
    wih1_own = din("wih1_own", [128, 48 * 128])
    wih1_oth = din("wih1_oth", [128, 48 * 128])
    sel0 = din("sel0", [128, 128])
    sel1 = din("sel1", [128, 128])
    ident = din("ident", [128, 128])
    attn_own = din("attn_own", [128, 32 * 128])
    attn_oth = din("attn_oth", [128, 32 * 128])
    fcw = din("fcw", [128, 8 * O])
    bias0 = din("bias0", [128, 12], F32)
    bias1 = din("bias1", [128, 12], F32)
    bhn0 = din("bhn0", [128, 32], F32)
    bhn1 = din("bhn1", [128, 32], F32)
    attn_b = din("attn_b", [128, 8], F32)
    fc_b = din("fc_b", [128, 1], F32)

    out_d = nc.dram_tensor("out", [O, TH, BL], F32, kind="ExternalOutput").ap()

    xg0d = nc.dram_tensor("xg0d", [128, 12, T * BL], F16).ap()
    xg1d = nc.dram_tensor("xg1d", [128, 12, T * BL], F16).ap()
    contrib0 = nc.dram_tensor("contrib0", [4, 128, T, BL], F16).ap()
    g0 = nc.dram_tensor("g0", [2, 4, 128, T, BL], F16).ap()
    contrib1 = nc.dram_tensor("contrib1", [4, 128, TH, BL], F16).ap()
    g1 = nc.dram_tensor("g1", [2, 4, 128, TH, BL], F16).ap()

    with ExitStack() as top:
        tc = top.enter_context(tile.TileContext(nc))

        const = top.enter_context(tc.tile_pool(name="const", bufs=1))
        # constants that live for the whole kernel
        sel0_sb = const.tile([128, 128], F16)
        sel1_sb = const.tile([128, 128], F16)
        ident_sb = const.tile([128, 128], F16)
        nc.sync.dma_start(sel0_sb[:], sel0[:])
        nc.sync.dma_start(sel1_sb[:], sel1[:])
        nc.sync.dma_start(ident_sb[:], ident[:])

        # ---------------- phase helpers ----------------
        def xg_phase(ctx, wih_tiles, nk, rhs_of_k, xgd, bias_ap, namep):
            """xg[m] = sum_k W[m,k] @ rhs_k  (+bias) -> xgd dram (fp16)."""
            sb = ctx.enter_context(tc.tile_pool(name=namep + "sb", bufs=4))
            ps = ctx.enter_context(
                tc.tile_pool(name=namep + "ps", bufs=2, space="PSUM"))
            for c in range(NCH):
                for m in range(12):
                    p = ps.tile([128, CH], F32, tag="xgps")
                    for k in range(nk):
                        nc.tensor.matmul(
                            p[:], wih_tiles(m, k), rhs_of_k(k, c),
                            start=(k == 0), stop=(k == nk - 1))
                    o = sb.tile([128, CH], F16, tag="xgsb")
                    if bias_ap is not None:
                        if m % 2 == 0:
                            nc.scalar.activation(o[:], p[:], AF.Identity,
                                                 bias=bias_ap[:, m:m + 1])
                        else:
                            nc.vector.tensor_scalar_add(o[:], p[:],
                                                        bias_ap[:, m:m + 1])
                    else:
                        if m % 2 == 0:
                            nc.scalar.copy(o[:], p[:])
                        else:
                            nc.vector.tensor_copy(o[:], p[:])
                    nc.sync.dma_start(xgd[:, m, c * CH:(c + 1) * CH], o[:])

        def recurrence(ctx, xgd, whh_sb, h_hist, bhn_ap, namep):
            hr = h_hist[:].rearrange("p (k t b) -> p k t b", k=4, t=T)
            xgp = ctx.enter_context(tc.tile_pool(name=namep + "xg", bufs=3))
            tmp = ctx.enter_context(tc.tile_pool(name=namep + "tmp", bufs=3))
            psr = ctx.enter_context(
                tc.tile_pool(name=namep + "psr", bufs=2, space="PSUM"))
            psn = ctx.enter_context(
                tc.tile_pool(name=namep + "psn", bufs=2, space="PSUM"))
            psz = ctx.enter_context(
                tc.tile_pool(name=namep + "psz", bufs=2, space="PSUM"))
            zpool = ctx.enter_context(tc.tile_pool(name=namep + "z", bufs=1))

            zrhs = zpool.tile([128, 32], F16)
            nc.vector.memset(zrhs[:], 0.0)

            xgc = None
            for t in range(T):
                if t % XB == 0:
                    xgc = xgp.tile([128, 12 * XB * BL], F16, tag="xgc")
                    nc.sync.dma_start(
                        xgc[:], xgd[:, :, t * BL:(t + XB) * BL])
                xv = xgc[:].rearrange("p (m t b) -> p m t b", m=12, t=XB)
                tl = t % XB
                hprev = (zrhs[:].rearrange("p (k b) -> p k b", k=4)
                         if t == 0 else hr[:, :, t - 1, :])

                def rhs(k):
                    if t == 0:
                        return zrhs[:, k * 8:(k + 1) * 8]
                    return hr[:, k, t - 1, :]

                def mm(ps_t, j, fold_xg):
                    jo = j % 4
                    for k in range(4):
                        nc.tensor.matmul(
                            ps_t[:, jo * 8:(jo + 1) * 8],
                            whh_sb[:, (j * 4 + k) * 128:(j * 4 + k + 1) * 128],
                            rhs(k), start=(k == 0),
                            stop=(k == 3 and not fold_xg))
                    if fold_xg:
                        nc.tensor.matmul(
                            ps_t[:, jo * 8:(jo + 1) * 8], ident_sb[:],
                            xv[:, j, tl, :], start=False, stop=True)

                pr = psr.tile([128, 32], F32, tag="pr")
                for j in range(0, 4):
                    mm(pr, j, True)
                rg = tmp.tile([128, 32], F32, tag="rg")
                nc.scalar.activation(rg[:], pr[:], AF.Sigmoid)

                pn = psn.tile([128, 32], F32, tag="pn")
                for j in range(4, 8):
                    mm(pn, j, False)
                if bhn_ap is not None:
                    t1_ = tmp.tile([128, 32], F32, tag="t1")
                    nc.vector.tensor_add(t1_[:], pn[:], bhn_ap)
                    nsrc = t1_
                else:
                    nsrc = pn
                t2 = tmp.tile([128, 32], F32, tag="t2")
                nc.vector.tensor_mul(t2[:], nsrc[:], rg[:])
                t3 = tmp.tile([128, 32], F32, tag="t3")
                nc.vector.tensor_add(t3[:].rearrange("p (m b) -> p m b", m=4),
                                     t2[:].rearrange("p (m b) -> p m b", m=4),
                                     xv[:, 4:8, tl, :])
                ng = tmp.tile([128, 32], F32, tag="ng")
                nc.scalar.activation(ng[:], t3[:], AF.Tanh)

                pz = psz.tile([128, 32], F32, tag="pz")
                for j in range(8, 12):
                    mm(pz, j, True)
                zg = tmp.tile([128, 32], F32, tag="zg")
                nc.scalar.activation(zg[:], pz[:], AF.Sigmoid)

                dd = tmp.tile([128, 32], F32, tag="dd")
                nc.vector.tensor_sub(dd[:].rearrange("p (k b) -> p k b", k=4),
                                     hprev, ng[:].rearrange(
                                         "p (k b) -> p k b", k=4))
                ee = tmp.tile([128, 32], F32, tag="ee")
                nc.vector.tensor_mul(ee[:], zg[:], dd[:])
                nc.vector.tensor_add(
                    hr[:, :, t, :],
                    ng[:].rearrange("p (k b) -> p k b", k=4),
                    ee[:].rearrange("p (k b) -> p k b", k=4))

        def exchange(h_hist, t_lo, t_cnt, contrib, gbuf):
            """contrib[i] = h_hist[t_lo + i] (forward); AllGather -> gbuf.
            Receivers un-reverse inside the select matmul's moving AP."""
            hr = h_hist[:].rearrange("p (k c) -> p k c", k=4)
            tch = 256 * BL
            ccnt = t_cnt * BL
            cfl = contrib[:].rearrange("k p t b -> k p (t b)")
            for k in range(4):
                src = hr[:, k, t_lo * BL: (t_lo + t_cnt) * BL]
                for q in range((ccnt + tch - 1) // tch):
                    s = slice(q * tch, min((q + 1) * tch, ccnt))
                    nc.sync.dma_start(cfl[k, :, s], src[:, s])
            nc.gpsimd.collective_compute(
                "AllGather", mybir.AluOpType.bypass,
                ins=[contrib[:]], outs=[gbuf[:]], replica_groups=GROUPS)

        def sel_other(ctx_pools, gbuf, nch, c, ch):
            """Select other-dir k-blocks for target chunk c (local time order).

            The gathered buffer holds the donor's hidden states in donor time
            order; local order is the full reversal, so source chunk is the
            mirrored one, read with a reversed-tau moving AP."""
            selsb, selps, hoth_pool = ctx_pools
            cs = nch - 1 - c                     # mirrored source chunk
            t0 = cs * (ch // BL)
            t1 = t0 + ch // BL
            hoth = []
            for kb in range(4):
                s0 = selsb.tile([128, ch], F16, tag="s0")
                nc.sync.dma_start(
                    s0[:], gbuf[0, kb].rearrange("p t b -> p (t b)")
                    [:, t0 * BL:t1 * BL])
                s1 = selsb.tile([128, ch], F16, tag="s1")
                nc.sync.dma_start(
                    s1[:], gbuf[1, kb].rearrange("p t b -> p (t b)")
                    [:, t0 * BL:t1 * BL])
                p = selps.tile([128, ch], F32, tag="selps")
                r0 = s0[:].rearrange("p (t b) -> p t b", b=BL)[:, ::-1, :]
                r1 = s1[:].rearrange("p (t b) -> p t b", b=BL)[:, ::-1, :]
                nc.tensor.matmul(p[:], sel0_sb[:], r0, start=True, stop=False)
                nc.tensor.matmul(p[:], sel1_sb[:], r1, start=False, stop=True)
                ho = hoth_pool.tile([128, ch], F16, tag="hoth")
                nc.vector.tensor_copy(ho[:], p[:])
                hoth.append(ho)
            return hoth

        # ---------------- phase 1: xg0 ----------------
        with ExitStack() as ctx:
            xsb = ctx.enter_context(tc.tile_pool(name="xsb", bufs=1))
            x_sb = xsb.tile([128, NCOL], F16)
            nc.sync.dma_start(x_sb[:], xt[:])
            wp = ctx.enter_context(tc.tile_pool(name="wih0p", bufs=1))
            wih0_sb = wp.tile([128, 12 * 128], F16)
            nc.sync.dma_start(wih0_sb[:], wih0[:])
            if with_bias[0]:
                b0p = ctx.enter_context(tc.tile_pool(name="b0p", bufs=1))
                b0_sb = b0p.tile([128, 12], F32)
                nc.sync.dma_start(b0_sb[:], bias0[:])
                b0_ap = b0_sb[:]
            else:
                b0_ap = None
            xg_phase(ctx,
                     lambda m, k: wih0_sb[:, m * 128:(m + 1) * 128],
                     1,
                     lambda k, c: x_sb[:, c * CH:(c + 1) * CH],
                     xg0d, b0_ap, "x0")

        # ---------------- phase 2: L0 recurrence ----------------
        h0_scope = ExitStack()
        h0p = h0_scope.enter_context(tc.tile_pool(name="h0p", bufs=1))
        h0_hist = h0p.tile([128, 4 * T * BL], F16)
        with ExitStack() as ctx:
            wp = ctx.enter_context(tc.tile_pool(name="whh0p", bufs=1))
            whh0_sb = wp.tile([128, 48 * 128], F16)
            nc.sync.dma_start(whh0_sb[:], whh0[:])
            bz = ctx.enter_context(tc.tile_pool(name="bhn0p", bufs=1))
            if with_bhn[0]:
                bhn0_sb = bz.tile([128, 32], F32)
                nc.sync.dma_start(bhn0_sb[:], bhn0[:])
                bhn_ap = bhn0_sb[:]
            else:
                bhn_ap = None
            recurrence(ctx, xg0d, whh0_sb, h0_hist, bhn_ap, "r0")

        # ---------------- phase 3: exchange h0 ----------------
        exchange(h0_hist, 0, T, contrib0, g0)

        # ---------------- phase 4: xg1 ----------------
        with ExitStack() as ctx:
            wp = ctx.enter_context(tc.tile_pool(name="wih1p", bufs=1))
            wih1o_sb = wp.tile([128, 48 * 128], F16, tag="wo")
            nc.sync.dma_start(wih1o_sb[:], wih1_own[:])
            wih1x_sb = wp.tile([128, 48 * 128], F16, tag="wx")
            nc.sync.dma_start(wih1x_sb[:], wih1_oth[:])
            b1p = ctx.enter_context(tc.tile_pool(name="b1p", bufs=1))
            if with_bias[1]:
                b1_sb = b1p.tile([128, 12], F32)
                nc.sync.dma_start(b1_sb[:], bias1[:])
                b1_ap = b1_sb[:]
            else:
                b1_ap = None
            selsb = ctx.enter_context(tc.tile_pool(name="sl4", bufs=3))
            selps = ctx.enter_context(
                tc.tile_pool(name="slp4", bufs=2, space="PSUM"))
            hop = ctx.enter_context(tc.tile_pool(name="ho4", bufs=8))
            sb = ctx.enter_context(tc.tile_pool(name="x1sb", bufs=4))
            ps = ctx.enter_context(
                tc.tile_pool(name="x1ps", bufs=2, space="PSUM"))
            h0r_flat = h0_hist
            for c in range(NCH):
                hoth = sel_other((selsb, selps, hop), g0, NCH, c, CH)
                for m in range(12):
                    p = ps.tile([128, CH], F32, tag="x1p")
                    for k in range(4):
                        nc.tensor.matmul(
                            p[:],
                            wih1o_sb[:, (m * 4 + k) * 128:(m * 4 + k + 1) * 128],
                            h0r_flat[:, k * NCOL + c * CH: k * NCOL + (c + 1) * CH],
                            start=(k == 0), stop=False)
                    for k in range(4):
                        nc.tensor.matmul(
                            p[:],
                            wih1x_sb[:, (m * 4 + k) * 128:(m * 4 + k + 1) * 128],
                            hoth[k][:], start=False, stop=(k == 3))
                    o = sb.tile([128, CH], F16, tag="x1o")
                    if b1_ap is not None:
                        if m % 2 == 0:
                            nc.scalar.activation(o[:], p[:], AF.Identity,
                                                 bias=b1_ap[:, m:m + 1])
                        else:
                            nc.vector.tensor_scalar_add(o[:], p[:],
                                                        b1_ap[:, m:m + 1])
                    else:
                        if m % 2 == 0:
                            nc.scalar.copy(o[:], p[:])
                        else:
                            nc.vector.tensor_copy(o[:], p[:])
                    nc.sync.dma_start(xg1d[:, m, c * CH:(c + 1) * CH], o[:])
        h0_scope.close()

        # ---------------- phase 5: L1 recurrence ----------------
        h1_scope = ExitStack()
        h1p = h1_scope.enter_context(tc.tile_pool(name="h1p", bufs=1))
        h1_hist = h1p.tile([128, 4 * T * BL], F16)
        with ExitStack() as ctx:
            wp = ctx.enter_context(tc.tile_pool(name="whh1p", bufs=1))
            whh1_sb = wp.tile([128, 48 * 128], F16)
            nc.sync.dma_start(whh1_sb[:], whh1[:])
            bz = ctx.enter_context(tc.tile_pool(name="bhn1p", bufs=1))
            if with_bhn[1]:
                bhn1_sb = bz.tile([128, 32], F32)
                nc.sync.dma_start(bhn1_sb[:], bhn1[:])
                bhn_ap = bhn1_sb[:]
            else:
                bhn_ap = None
            recurrence(ctx, xg1d, whh1_sb, h1_hist, bhn_ap, "r1")

        # ---------------- phase 6: exchange h1 tail ----------------
        exchange(h1_hist, TH, TH, contrib1, g1)

        # ---------------- phase 7: attention + fc ----------------
        with ExitStack() as ctx:
            wp = ctx.enter_context(tc.tile_pool(name="awp", bufs=1))
            attno_sb = wp.tile([128, 32 * 128], F16, tag="ao")
            nc.sync.dma_start(attno_sb[:], attn_own[:])
            attnx_sb = wp.tile([128, 32 * 128], F16, tag="ax")
            nc.sync.dma_start(attnx_sb[:], attn_oth[:])
            fcw_sb = wp.tile([128, 8 * O], F16, tag="fw")
            nc.sync.dma_start(fcw_sb[:], fcw[:])
            ab_sb = wp.tile([128, 8], F32, tag="ab")
            if with_attn_bias:
                nc.sync.dma_start(ab_sb[:], attn_b[:])
            fb_sb = wp.tile([128, 1], F32, tag="fb")
            if with_fc_bias:
                nc.sync.dma_start(fb_sb[:], fc_b[:])

            selsb = ctx.enter_context(tc.tile_pool(name="sl7", bufs=3))
            selps = ctx.enter_context(
                tc.tile_pool(name="slp7", bufs=2, space="PSUM"))
            hop = ctx.enter_context(tc.tile_pool(name="ho7", bufs=8))
            sb = ctx.enter_context(tc.tile_pool(name="asb", bufs=4))
            aps = ctx.enter_context(
                tc.tile_pool(name="aps", bufs=2, space="PSUM"))
            fps = ctx.enter_context(
                tc.tile_pool(name="fps", bufs=2, space="PSUM"))
            for c in range(NCH2):
                hoth = sel_other((selsb, selps, hop), g1, NCH2, c, CH2)
                pf = fps.tile([O, CH2], F32, tag="fcp")
                for m in range(8):
                    p = aps.tile([128, CH2], F32, tag="ap")
                    for k in range(4):
                        nc.tensor.matmul(
                            p[:],
                            attno_sb[:, (m * 4 + k) * 128:(m * 4 + k + 1) * 128],
                            h1_hist[:, k * NCOL + c * CH2: k * NCOL + (c + 1) * CH2],
                            start=(k == 0), stop=False)
                    for k in range(4):
                        nc.tensor.matmul(
                            p[:],
                            attnx_sb[:, (m * 4 + k) * 128:(m * 4 + k + 1) * 128],
                            hoth[k][:], start=False, stop=(k == 3))
                    at = sb.tile([128, CH2], F32, tag="at")
                    if with_attn_bias:
                        nc.scalar.activation(at[:], p[:], AF.Tanh,
                                             bias=ab_sb[:, m:m + 1])
                    else:
                        nc.scalar.activation(at[:], p[:], AF.Tanh)
                    gt = sb.tile([128, CH2], F16, tag="gt")
                    if m < 4:
                        hloc = h1_hist[:, m * NCOL + c * CH2: m * NCOL + (c + 1) * CH2]
                    else:
                        hloc = hoth[m - 4][:]
                    nc.vector.tensor_mul(gt[:], at[:], hloc)
                    nc.tensor.matmul(pf[:], fcw_sb[:, m * O:(m + 1) * O], gt[:],
                                     start=(m == 0), stop=(m == 7))
                ot = sb.tile([O, CH2], F32, tag="ot")
                if with_fc_bias:
                    nc.scalar.activation(ot[:], pf[:], AF.Identity,
                                         bias=fb_sb[0:O, 0:1])
                else:
                    nc.scalar.copy(ot[:], pf[:])
                t0 = c * (CH2 // BL)
                t1 = (c + 1) * (CH2 // BL)
                nc.sync.dma_start(out_d[:, t0:t1, :], ot[:])
        h1_scope.close()

    nc.compile()
    return nc


# ----------------------------------------------------------------- host prep
def prep_core_inputs(inputs, c, T=T_FULL):
    d, g = c % 2, c // 2
    TH = T // 2
    f16 = lambda a: np.ascontiguousarray(a, dtype=np.float16)
    f32 = lambda a: np.ascontiguousarray(a, dtype=np.float32)

    x = np.asarray(inputs['x'])[g * BL:(g + 1) * BL, :T]      # [8, T, 128]
    if d == 1:
        x = x[:, ::-1]
    xt = f16(x.transpose(2, 1, 0).reshape(128, T * BL))

    w_hh0 = np.asarray(inputs['W_hh0'])[d]     # [1536, 512]
    w_hh1 = np.asarray(inputs['W_hh1'])[d]
    w_ih0 = np.asarray(inputs['W_ih0'])[d]     # [1536, 128]
    w_ih1 = np.asarray(inputs['W_ih1'])[d]     # [1536, 1024]
    b_ih0 = np.asarray(inputs['b_ih0'])[d]
    b_hh0 = np.asarray(inputs['b_hh0'])[d]
    b_ih1 = np.asarray(inputs['b_ih1'])[d]
    b_hh1 = np.asarray(inputs['b_hh1'])[d]
    attn_W = np.asarray(inputs['attn_W'])      # [1024, 1024]
    attn_bv = np.asarray(inputs['attn_b'])
    fc_W = np.asarray(inputs['fc_W'])          # [10, 1024]
    fc_bv = np.asarray(inputs['fc_b'])

    def whh_tiles(w):
        out = np.zeros((128, 48 * 128), np.float16)
        for j in range(12):
            rb = PERMROWS[j]
            for k in range(4):
                blk = w[rb * 128:(rb + 1) * 128, k * 128:(k + 1) * 128]
                out[:, (j * 4 + k) * 128:(j * 4 + k + 1) * 128] = \
                    blk.T.astype(np.float16)
        return out

    whh0 = whh_tiles(w_hh0)
    whh1 = whh_tiles(w_hh1)

    wih0 = np.zeros((128, 12 * 128), np.float16)
    for j in range(12):
        rb = PERMROWS[j]
        wih0[:, j * 128:(j + 1) * 128] = \
            w_ih0[rb * 128:(rb + 1) * 128, :].T.astype(np.float16)

    own_lo = 0 if d == 0 else 512
    oth_lo = 512 - own_lo

    def wih1_tiles(col_lo):
        out = np.zeros((128, 48 * 128), np.float16)
        for j in range(12):
            rb = PERMROWS[j]
            for k in range(4):
                blk = w_ih1[rb * 128:(rb + 1) * 128,
                            col_lo + k * 128: col_lo + (k + 1) * 128]
                out[:, (j * 4 + k) * 128:(j * 4 + k + 1) * 128] = \
                    blk.T.astype(np.float16)
        return out

    wih1_own = wih1_tiles(own_lo)
    wih1_oth = wih1_tiles(oth_lo)

    identm = np.eye(128, dtype=np.float16)
    zer = np.zeros((128, 128), np.float16)
    sel0 = identm if d == 1 else zer      # gathered rank0 = fwd core
    sel1 = identm if d == 0 else zer

    # attention: rows and cols in LOCAL order (own dims first)
    attn_local = np.concatenate(
        [attn_W[own_lo:own_lo + 512], attn_W[oth_lo:oth_lo + 512]], axis=0)

    def attn_tiles(col_lo):
        out = np.zeros((128, 32 * 128), np.float16)
        for m in range(8):
            for k in range(4):
                blk = attn_local[m * 128:(m + 1) * 128,
                                 col_lo + k * 128: col_lo + (k + 1) * 128]
                out[:, (m * 4 + k) * 128:(m * 4 + k + 1) * 128] = \
                    blk.T.astype(np.float16)
        return out

    attn_own = attn_tiles(own_lo)
    attn_oth = attn_tiles(oth_lo)

    fc_local = np.concatenate(
        [fc_W[:, own_lo:own_lo + 512], fc_W[:, oth_lo:oth_lo + 512]], axis=1)
    fcw = np.zeros((128, 8 * O), np.float16)
    for k in range(8):
        fcw[:, k * O:(k + 1) * O] = \
            fc_local[:, k * 128:(k + 1) * 128].T.astype(np.float16)

    # biases: fold b_ih + b_hh(r,z) into xg bias; n keeps b_ih only + bhn tile
    def gate_bias(b_ih, b_hh):
        v = b_ih.astype(np.float64).copy()
        v[:H] += b_hh[:H]              # r
        v[H:2 * H] += b_hh[H:2 * H]    # z
        bias = np.zeros((128, 12), np.float32)
        for j in range(12):
            rb = PERMROWS[j]
            bias[:, j] = v[rb * 128:(rb + 1) * 128]
        return bias

    bias0 = gate_bias(b_ih0, b_hh0)
    bias1 = gate_bias(b_ih1, b_hh1)
    bhn0 = np.zeros((128, 32), np.float32)
    bhn1 = np.zeros((128, 32), np.float32)
    for jj in range(4):
        bhn0[:, jj * 8:(jj + 1) * 8] = \
            b_hh0[2 * H + jj * 128: 2 * H + (jj + 1) * 128, None]
        bhn1[:, jj * 8:(jj + 1) * 8] = \
            b_hh1[2 * H + jj * 128: 2 * H + (jj + 1) * 128, None]

    attn_b_local = np.concatenate(
        [attn_bv[own_lo:own_lo + 512], attn_bv[oth_lo:oth_lo + 512]])
    attn_b = np.zeros((128, 8), np.float32)
    for m in range(8):
        attn_b[:, m] = attn_b_local[m * 128:(m + 1) * 128]
    fc_b = np.zeros((128, 1), np.float32)
    fc_b[:O, 0] = fc_bv

    return {
        "xt": xt, "whh0": whh0, "whh1": whh1, "wih0": wih0,
        "wih1_own": wih1_own, "wih1_oth": wih1_oth,
        "sel0": sel0, "sel1": sel1, "ident": identm,
        "attn_own": attn_own, "attn_oth": attn_oth, "fcw": fcw,
        "bias0": f32(bias0), "bias1": f32(bias1),
        "bhn0": f32(bhn0), "bhn1": f32(bhn1),
        "attn_b": f32(attn_b), "fc_b": f32(fc_b),
    }


def flags_from_inputs(inputs):
    nz = lambda a: bool(np.any(np.asarray(a)))
    with_bhn = (nz(np.asarray(inputs['b_hh0'])[:, 2 * H:]),
                nz(np.asarray(inputs['b_hh1'])[:, 2 * H:]))
    with_bias = (nz(inputs['b_ih0']) or nz(np.asarray(inputs['b_hh0'])[:, :2 * H]),
                 nz(inputs['b_ih1']) or nz(np.asarray(inputs['b_hh1'])[:, :2 * H]))
    return dict(with_bhn=with_bhn, with_bias=with_bias,
                with_attn_bias=nz(inputs['attn_b']),
                with_fc_bias=nz(inputs['fc_b']))


_PROG_CACHE = {}


def _get_program(T, flags):
    key = (T, tuple(sorted((k, tuple(v) if isinstance(v, tuple) else v)
                           for k, v in flags.items())))
    if key not in _PROG_CACHE:
        _PROG_CACHE[key] = build_program(T=T, **flags)
    return _PROG_CACHE[key]


def run_cores(inputs, T=T_FULL, trace=False, **kw):
    flags = flags_from_inputs(inputs)
    nc = _get_program(T, flags)
    in_maps = [prep_core_inputs(inputs, c, T=T) for c in range(N_CORES)]
    res = run_bass_kernel_spmd(nc, in_maps, list(range(N_CORES)), trace=trace,
                               **kw)
    return res


def assemble_output(results, T=T_FULL):
    TH = T // 2
    out = np.zeros((B, T, O), np.float32)
    for c in range(N_CORES):
        d, g = c % 2, c // 2
        r = results[c]["out"].transpose(2, 1, 0)   # [O,TH,BL] -> [BL,TH,O]
        if d == 0:
            out[g * BL:(g + 1) * BL, :TH] = r
        else:
            out[g * BL:(g + 1) * BL, TH:] = r[:, ::-1, :]
    return out


def kernel(**inputs) -> np.ndarray:
    res = run_cores(inputs, T=T_FULL)
    return assemble_output(res.results, T=T_FULL)


if __name__ == "__main__":
    pass
